# revision 1
# baseline (speedup 1.0000x reference)
"""DyGraphGIN2d Trainium kernel: kNN graph (k=16) + GIN aggregation + MLP/BN/GELU.

Sharding: data-parallel over batch B=8 across 8 NeuronCores (one batch
element per core; kNN graphs are per-element independent). BatchNorm uses
training-mode batch statistics over ALL B*N rows, so the per-core partial
sums (a [64,2] tensor) go through one in-kernel AllReduce.

Algorithm per core (N=4096 points, C=64 dims):
  Phase 1 (threshold): ranking value s'[n,m] = <x_n,x_m> - |x_m|^2/2
    (distance-order-equivalent) is computed per 128-row stripe via three
    f32r matmuls: x = x_hi + x_lo is an EXACT split (f32r keeps 11
    explicit mantissa bits, so hi*hi/hi*lo/lo*hi products are exact and
    only lo*lo ~2^-24 is dropped) at 1 cy/row instead of fp32's 4 cy/row.
    DVE `max` (top-8) over 256-wide chunks + max/match_replace/max over
    the 128 candidates gives each row's exact 16th-largest value tau.
  Phase 2 (mask + aggregate): v[m,n] is recomputed in transposed
    orientation with the SAME mirrored 3-matmul sequence, making v ==
    s'.T bit-exact; mask = (v >= tau[n]) on DVE selects exactly the k
    nearest neighbors (self included). aggr[c,n] = x^T @ mask accumulates
    in PSUM via bf16 matmuls (mask is exactly 0/1 in bf16).
  Tail: h = (1+eps)x + aggr; h1 = W1^T h + b1; BN stats sum/sumsq ->
    AllReduce over 8 cores -> fused BN+erf-GELU in one ACT pass
    (gelu(h1*scale + shift) with per-partition scale/bias); out = W2^T hg
    + b2 -> DMA out in [O, N] layout per core.

The jitted 8-core shard_map executable is cached across kernel() calls.
"""

import numpy as np
import ml_dtypes

import concourse.bacc as bacc
import concourse.mybir as mybir
from concourse.tile import TileContext

F32 = mybir.dt.float32
F32R = mybir.dt.float32r
BF16 = mybir.dt.bfloat16
AF = mybir.ActivationFunctionType
ALU = mybir.AluOpType

B, C, N, O = 8, 64, 4096, 64
K_NN = 16
N_CORES = 8
NT = N // 128          # 32 row tiles
BN_EPS = 1e-5
BN_COUNT = float(B * N)

_cache = {}


def _build():
    nc = bacc.Bacc("TRN2", target_bir_lowering=False)

    xb_d = nc.dram_tensor("xb", [C, N], F32, kind="ExternalInput")
    id_d = nc.dram_tensor("ident64", [C, C], BF16, kind="ExternalInput")
    w1_d = nc.dram_tensor("w1", [C, O], F32, kind="ExternalInput")
    w2_d = nc.dram_tensor("w2", [O, O], F32, kind="ExternalInput")
    vecs_d = nc.dram_tensor("vecs", [O, 5], F32, kind="ExternalInput")  # b1,gamma,beta,b2,eps1
    ones_r_d = nc.dram_tensor("ones_row", [1, N], F32R, kind="ExternalInput")
    ones_c_d = nc.dram_tensor("ones_col", [C, 1], F32, kind="ExternalInput")
    y_d = nc.dram_tensor("y", [O, N], F32, kind="ExternalOutput")

    tau_dram_a = nc.dram_tensor("tau_scratch_a", [N // 2, 1], F32)  # internal
    tau_dram_b = nc.dram_tensor("tau_scratch_b", [N // 2, 1], F32)  # internal

    with TileContext(nc) as tc:
        with tc.tile_pool(name="big", bufs=1) as big, \
             tc.tile_pool(name="work", bufs=1) as work, \
             tc.tile_pool(name="dram", bufs=1, space="DRAM") as dpool:

            # ---- operand prep: exact hi/lo f32r decomposition ----------
            # f32r keeps 11 explicit mantissa bits; x = x_hi + x_lo with both
            # f32r-exact, so hi*hi + hi*lo + lo*hi are EXACT products (fp32
            # PSUM accumulate) and only lo*lo (~2^-24) is dropped. Three f32r
            # matmuls at 1 cy/row replace one fp32 matmul at 4 cy/row.
            # Ranking value: s' = <x_n, x_m> - sq_m/2 (distance-equivalent).
            # Aug rows: XH1 = [x_hi; 1], XH2 = [x_hi; q_hi], XLO = [x_lo; q_lo]
            # with q_hi + q_lo = -sq/2 split the same way.
            xb_sb = big.tile([64, N], F32)
            nc.sync.dma_start(xb_sb[:, :], xb_d[:, :])
            XH1 = big.tile([128, N], F32R)
            # XH2/XLO as 8 per-chunk column tiles so phase-1 matmuls on chunk
            # c only wait for chunk c's q-row (whole-tile deps otherwise
            # serialize ~30us of prep before the first stripe).
            XH2c = [big.tile([128, 512], F32R, name=f"XH2c{i}") for i in range(8)]
            XLOc = [big.tile([128, 512], F32R, name=f"XLOc{i}") for i in range(8)]
            for c8 in range(8):
                sl = slice(c8 * 512, (c8 + 1) * 512)
                nc.scalar.activation(XH1[:C, sl], xb_sb[:, sl], AF.Copy)
            nc.sync.dma_start(XH1[C : C + 1, :], ones_r_d[:, :])
            ones_col = work.tile([128, 1], F32)
            nc.sync.dma_start(ones_col[:C, :], ones_c_d[:, :])
            lo_t = work.tile([64, 512], F32, tag="lo_t", bufs=3)
            xsq = work.tile([64, 512], F32, tag="xsq", bufs=3)
            qtmp = work.tile([1, 512], F32, tag="qtmp", bufs=2)
            ps_qh = work
            with tc.tile_pool(name="ps_sq", bufs=2, space="PSUM") as ps_sq:
              for c8 in range(8):
                sl = slice(c8 * 512, (c8 + 1) * 512)
                nc.sync.dma_start(XH2c[c8][:C, :], XH1[:C, sl])
                nc.vector.tensor_tensor(out=lo_t[:, :], in0=xb_sb[:, sl],
                                        in1=XH1.bitcast(F32)[:C, sl],
                                        op=ALU.subtract)
                nc.vector.tensor_copy(XLOc[c8][:C, :], lo_t[:, :])
                nc.vector.tensor_tensor(out=xsq[:, :], in0=xb_sb[:, sl],
                                        in1=xb_sb[:, sl], op=ALU.mult)
                sq_ps = ps_sq.tile([1, 512], F32, tag="sq_ps")
                nc.tensor.matmul(sq_ps[:, :], ones_col[:C, :], xsq[:, :],
                                 start=True, stop=True)
                # q_hi straight into XH2 row 64 (ACT handles base-64 out),
                # plus a partition-0 twin for the exact q_lo subtraction.
                qh = ps_qh.tile([1, 512], F32R, tag="qh", bufs=2)
                nc.scalar.activation(XH2c[c8][C : C + 1, :], sq_ps[:, :], AF.Copy,
                                     scale=-0.5)
                nc.scalar.activation(qh[:, :], sq_ps[:, :], AF.Copy, scale=-0.5)
                nc.vector.tensor_scalar(out=qtmp[:, :], in0=sq_ps[:, :],
                                        scalar1=-0.5, scalar2=None, op0=ALU.mult)
                nc.vector.tensor_tensor(out=qtmp[:, :], in0=qtmp[:, :],
                                        in1=qh.bitcast(F32)[:, :],
                                        op=ALU.subtract)
                nc.scalar.activation(XLOc[c8][C : C + 1, :], qtmp[:, :], AF.Copy)

            # ---- phase 1: per-row 16th-largest threshold ----------------
            cand = work.tile([128, 128], F32, tag="cand", bufs=3)
            t8a = work.tile([128, 8], F32, tag="t8a", bufs=3)
            t8b = work.tile([128, 8], F32, tag="t8b", bufs=3)
            tau = work.tile([128, 1], F32, tag="tau", bufs=3)

            h_sb = big.tile([64, N], F32, tag="h_shared")  # assembled h
            eps1 = work.tile([64, 1], F32)
            nc.sync.dma_start(eps1[:, :], vecs_d[:, 4:5])
            tau_bc_a = big.tile([128, N // 2], F32)
            tau_bc_b = big.tile([128, N // 2], F32)
            mask_b = work.tile([128, 2048], BF16, tag="mask", bufs=3)

            ps_sv_cm = tc.tile_pool(name="ps_sv", bufs=4, space="PSUM")
            ps_sv = ps_sv_cm.__enter__()
            ps_s = ps_v = ps_sv
            _cms = [ps_sv_cm]

            def stripe(j):
                jj = slice(j * 128, (j + 1) * 128)
                for c8 in range(8):
                    s_ps = ps_s.tile([128, 512], F32, tag="sv_ps", name=f"s_ps_{j}_{c8}")
                    jt, jo = j // 4, (j % 4) * 128
                    nc.tensor.matmul(s_ps[:, :], XH1[: C + 1, jj],
                                     XH2c[c8][: C + 1, :], start=True, stop=False)
                    nc.tensor.matmul(s_ps[:, :], XH1[: C + 1, jj],
                                     XLOc[c8][: C + 1, :], start=False, stop=False)
                    nc.tensor.matmul(s_ps[:, :], XLOc[jt][:C, jo : jo + 128],
                                     XH2c[c8][:C, :], start=False, stop=True)
                    for h in range(2):
                        nc.vector.max(
                            out=cand[:, (c8 * 2 + h) * 8 : (c8 * 2 + h + 1) * 8],
                            in_=s_ps[:, h * 256 : (h + 1) * 256])
                nc.vector.max(out=t8a[:, :], in_=cand[:, :])
                nc.vector.match_replace(out=cand[:, :], in_to_replace=t8a[:, :],
                                        in_values=cand[:, :], imm_value=-1e30)
                nc.vector.max(out=t8b[:, :], in_=cand[:, :])
                # phase-2 recomputes s bit-exactly; tiny guard as insurance
                nc.vector.tensor_scalar(out=tau[:, :], in0=t8b[:, 7:8],
                                        scalar1=1e-6, scalar2=None,
                                        op0=ALU.subtract)
                if j < NT // 2:
                    nc.sync.dma_start(tau_dram_a[jj, :], tau[:, :])
                else:
                    nc.sync.dma_start(
                        tau_dram_b[(j - NT // 2) * 128 : (j - NT // 2 + 1) * 128, :],
                        tau[:, :])

            aggr_tiles = {}

            def p2_block(H, j):
                # one (H, j) unit: 8 f32r matmul thirds -> 4 masks -> 4 aggr
                aggr_ps = aggr_tiles[H]
                tb = tau_bc_a if H == 0 else tau_bc_b
                for c4 in range(4):
                    nsl = slice(H * 2048 + c4 * 512, H * 2048 + (c4 + 1) * 512)
                    v_ps = ps_v.tile([128, 512], F32, tag="sv_ps", name=f"v_ps_{H}_{j}_{c4}")
                    jt, jo = j // 4, (j % 4) * 128
                    nct = (H * 2048 + c4 * 512) // 512
                    nc.tensor.matmul(v_ps[:, :], XH2c[jt][: C + 1, jo : jo + 128],
                                     XH1[: C + 1, nsl], start=True, stop=False)
                    nc.tensor.matmul(v_ps[:, :], XLOc[jt][: C + 1, jo : jo + 128],
                                     XH1[: C + 1, nsl], start=False, stop=False)
                    nc.tensor.matmul(v_ps[:, :], XH2c[jt][:C, jo : jo + 128],
                                     XLOc[nct][:C, :], start=False, stop=True)
                    nc.vector.tensor_tensor(
                        out=mask_b[:, c4 * 512 : (c4 + 1) * 512],
                        in0=v_ps[:, :], in1=tb[:, c4 * 512 : (c4 + 1) * 512],
                        op=ALU.is_ge)
                for c4 in range(4):
                    nc.tensor.matmul(
                        aggr_ps[:, c4 * 512 : (c4 + 1) * 512],
                        xt_sb[:, j * C : (j + 1) * C],
                        mask_b[:, c4 * 512 : (c4 + 1) * 512],
                        start=(j == 0), stop=(j == NT - 1))

            def finish_half(H):
                hh = slice(H * 2048, (H + 1) * 2048)
                nc.vector.tensor_scalar(out=h_sb[:, hh], in0=xb_sb[:, hh],
                                        scalar1=eps1[:, :], scalar2=None, op0=ALU.mult)
                nc.vector.tensor_tensor(out=h_sb[:, hh], in0=h_sb[:, hh],
                                        in1=aggr_tiles[H][:, :], op=ALU.add)

            # stripes 0..15 (tau half a), then stripes 16..31 interleaved
            # with phase-2 H=0 blocks, then phase-2 H=1.
            for j in range(NT // 2):
                stripe(j)

            # xt16 (bf16 transposed x), emitted after the phase-1 prefix so
            # its PE/DVE stream slots don't delay the first stripes; copies
            # go to the otherwise-idle ACT engine.
            xb16 = work.tile([64, N], BF16)
            nc.vector.tensor_copy(xb16[:, :], xb_sb[:, :])
            ident = work.tile([64, C], BF16)
            nc.sync.dma_start(ident[:, :], id_d[:, :])
            xt_sb = work.tile([128, NT * C], BF16)
            with tc.tile_pool(name="ps_tp", bufs=2, space="PSUM") as ps_tp:
                for j in range(NT):
                    tp_ps = ps_tp.tile([128, C], BF16, tag="tp_ps")
                    nc.tensor.transpose(tp_ps[:, :],
                                        xb16[:, j * 128 : (j + 1) * 128],
                                        ident[:, :])
                    nc.scalar.activation(xt_sb[:, j * C : (j + 1) * C],
                                         tp_ps[:, :], AF.Copy)

            nc.sync.dma_start(
                tau_bc_a[:, :],
                tau_dram_a[:, 0:1].rearrange("m one -> one m").to_broadcast([128, N // 2]))
            ps_aggr_cm = tc.tile_pool(name="ps_aggr", bufs=1, space="PSUM")
            ps_aggr = ps_aggr_cm.__enter__()
            _cms.append(ps_aggr_cm)
            aggr_tiles[0] = ps_aggr.tile([64, 2048], F32, tag="aggr_ps", name="aggr0")
            for t in range(NT // 2):
                stripe(NT // 2 + t)
                p2_block(0, 2 * t)
                p2_block(0, 2 * t + 1)
            nc.sync.dma_start(
                tau_bc_b[:, :],
                tau_dram_b[:, 0:1].rearrange("m one -> one m").to_broadcast([128, N // 2]))
            finish_half(0)

            # W1 + BN partial stats for half 0, emitted inside the (PE-bound,
            # DVE/ACT-idle) H=1 segment so only half 1's stats + the
            # collective remain serial at the tail.
            w1_sb = work.tile([64, O], F32)
            w2_sb = work.tile([64, O], F32)
            vecs_sb = work.tile([64, 5], F32)
            nc.sync.dma_start(w1_sb[:, :], w1_d[:, :])
            nc.sync.dma_start(w2_sb[:, :], w2_d[:, :])
            nc.sync.dma_start(vecs_sb[:, :], vecs_d[:, :])
            h1_sb = big.tile([64, N], F32)
            sq_scratch = big.tile([64, N], F32, tag="hg_shared")
            stats_h = work.tile([64, 4], F32)  # cols: sum0, sumsq0, sum1, sumsq1

            def mlp_half(H):
                for c4 in range(4):
                    sl = slice(H * 2048 + c4 * 512, H * 2048 + (c4 + 1) * 512)
                    h1_ps = ps_v.tile([64, 512], F32, tag="sv_ps",
                                      name=f"h1_ps_{H}_{c4}")
                    nc.tensor.matmul(h1_ps[:, :], w1_sb[:, :], h_sb[:, sl],
                                     start=True, stop=True)
                    nc.vector.tensor_scalar(out=h1_sb[:, sl], in0=h1_ps[:, :],
                                            scalar1=vecs_sb[:, 0:1], scalar2=None,
                                            op0=ALU.add)
                hh = slice(H * 2048, (H + 1) * 2048)
                nc.vector.reduce_sum(stats_h[:, 2 * H : 2 * H + 1], h1_sb[:, hh],
                                     axis=mybir.AxisListType.X)
                nc.scalar.activation(sq_scratch[:, hh], h1_sb[:, hh], AF.Square,
                                     accum_out=stats_h[:, 2 * H + 1 : 2 * H + 2])

            aggr_tiles[1] = ps_aggr.tile([64, 2048], F32, tag="aggr_ps", name="aggr1")
            for j in range(NT // 2):
                p2_block(1, j)
            mlp_half(0)
            for j in range(NT // 2, NT):
                p2_block(1, j)
            finish_half(1)
            mlp_half(1)
            for cm in reversed(_cms):
                cm.__exit__(None, None, None)

            # ---- BN combine + GELU + W2 ---------------------------------
            ps_mlp_cm = tc.tile_pool(name="ps_mlp", bufs=4, space="PSUM")
            ps_mlp = ps_mlp_cm.__enter__()
            stats = work.tile([64, 2], F32)
            nc.vector.tensor_tensor(out=stats[:, :], in0=stats_h[:, 0:2],
                                    in1=stats_h[:, 2:4], op=ALU.add)

            cc_in = dpool.tile([64, 2], F32)
            cc_out = dpool.tile([64, 2], F32, addr_space="Shared")
            nc.sync.dma_start(cc_in[:, :], stats[:, :])
            nc.gpsimd.collective_compute(
                "AllReduce", ALU.add,
                ins=[cc_in[:, :]],
                outs=[cc_out[:, :]],
                replica_groups=[list(range(N_CORES))],
            )
            gstats = work.tile([64, 2], F32)
            nc.sync.dma_start(gstats[:, :], cc_out[:, :])

            # mean/var -> scale/shift  (all [64,1] minis)
            mean = work.tile([64, 1], F32)
            var = work.tile([64, 1], F32)
            scale = work.tile([64, 1], F32)
            shift = work.tile([64, 1], F32)
            tmp = work.tile([64, 1], F32)
            nc.vector.tensor_scalar(out=mean[:, :], in0=gstats[:, 0:1],
                                    scalar1=1.0 / BN_COUNT, scalar2=None, op0=ALU.mult)
            nc.vector.tensor_scalar(out=var[:, :], in0=gstats[:, 1:2],
                                    scalar1=1.0 / BN_COUNT, scalar2=None, op0=ALU.mult)
            nc.vector.tensor_tensor(out=tmp[:, :], in0=mean[:, :], in1=mean[:, :],
                                    op=ALU.mult)
            nc.vector.tensor_tensor(out=var[:, :], in0=var[:, :], in1=tmp[:, :],
                                    op=ALU.subtract)
            # rstd = 1/sqrt(var + eps)
            nc.vector.tensor_scalar(out=var[:, :], in0=var[:, :], scalar1=BN_EPS,
                                    scalar2=None, op0=ALU.add)
            nc.scalar.activation(tmp[:, :], var[:, :], AF.Sqrt)
            nc.vector.reciprocal(out=tmp[:, :], in_=tmp[:, :])
            nc.vector.tensor_tensor(out=scale[:, :], in0=vecs_sb[:, 1:2],
                                    in1=tmp[:, :], op=ALU.mult)  # gamma * rstd
            nc.vector.tensor_tensor(out=tmp[:, :], in0=mean[:, :], in1=scale[:, :],
                                    op=ALU.mult)
            nc.vector.tensor_tensor(out=shift[:, :], in0=vecs_sb[:, 2:3],
                                    in1=tmp[:, :], op=ALU.subtract)  # beta - mean*scale

            # fused BN + GELU on ACT: gelu(h1*scale + shift)
            hg = big.tile([64, N], F32, tag="hg_shared")
            nc.scalar.activation(hg[:, :], h1_sb[:, :], AF.Gelu,
                                 scale=scale[:, :], bias=shift[:, :])

            # out = w2^T hg + b2 -> y
            y_sb = big.tile([64, N], F32, tag="h_shared")
            for c8 in range(8):
                sl = slice(c8 * 512, (c8 + 1) * 512)
                o_ps = ps_mlp.tile([64, 512], F32, tag="h1_ps")
                nc.tensor.matmul(o_ps[:, :], w2_sb[:, :], hg[:, sl],
                                 start=True, stop=True)
                nc.vector.tensor_scalar(out=y_sb[:, sl], in0=o_ps[:, :],
                                        scalar1=vecs_sb[:, 3:4], scalar2=None,
                                        op0=ALU.add)
                nc.sync.dma_start(y_d[:, sl], y_sb[:, sl])
            ps_mlp_cm.__exit__(None, None, None)

    if not nc.is_finalized():
        nc.finalize()
    return nc


def _get_runner():
    """Build the Bass module once and cache a jitted 8-core executable.

    Mirrors bass2jax.run_bass_via_pjrt's multi-core path, but keeps the
    jitted shard_map callable across invocations (run_bass_via_pjrt
    rebuilds and retraces it per call, which costs hundreds of ms).
    """
    if "runner" in _cache:
        return _cache["runner"]

    import jax
    import concourse.mybir as mb
    from jax.sharding import Mesh, PartitionSpec
    from jax.experimental.shard_map import shard_map
    from concourse import bass2jax

    nc = _build()
    bass2jax.install_neuronx_cc_hook()

    partition_name = nc.partition_id_tensor.name if nc.partition_id_tensor else None
    in_names = []
    out_names = []
    out_avals = []
    for alloc in nc.m.functions[0].allocations:
        if not isinstance(alloc, mb.MemoryLocationSet):
            continue
        name = alloc.memorylocations[0].name
        if alloc.kind == "ExternalInput":
            if name != partition_name:
                in_names.append(name)
        elif alloc.kind == "ExternalOutput":
            out_names.append(name)
            out_avals.append(
                jax.core.ShapedArray(tuple(alloc.tensor_shape), mb.dt.np(alloc.dtype))
            )
    n_params = len(in_names)
    all_in_names = list(in_names)
    if partition_name is not None:
        all_in_names = all_in_names + [partition_name]

    def _body(*args):
        # No zero output operands: the kernel writes every output element,
        # so uninitialized custom-call result buffers are fine.
        operands = list(args)
        if partition_name is not None:
            operands.append(bass2jax.partition_id_tensor())
        outs = bass2jax._bass_exec_p.bind(
            *operands,
            out_avals=tuple(out_avals),
            in_names=tuple(all_in_names),
            out_names=tuple(out_names),
            lowering_input_output_aliases=(),
            sim_require_finite=True,
            sim_require_nnan=True,
            nc=nc,
        )
        return tuple(outs)

    devices = jax.devices()[:N_CORES]
    assert len(devices) == N_CORES, f"need {N_CORES} devices, have {len(jax.devices())}"
    mesh = Mesh(np.asarray(devices), ("core",))
    n_outs = len(out_names)
    sharded = jax.jit(
        shard_map(
            _body,
            mesh=mesh,
            in_specs=(PartitionSpec("core"),) * n_params,
            out_specs=(PartitionSpec("core"),) * n_outs,
            check_rep=False,
        ),
        keep_unused=True,
    )
    _cache["runner"] = (sharded, in_names, out_names, out_avals)
    return _cache["runner"]


def kernel(**inputs) -> np.ndarray:
    x = np.asarray(inputs["x"], dtype=np.float32)
    assert x.shape == (B, C, N, 1), x.shape
    k = int(np.asarray(inputs.get("k", K_NN)))
    assert k == K_NN, f"kernel compiled for k={K_NN}, got {k}"
    w1 = np.asarray(inputs["w1"], dtype=np.float32)
    b1 = np.asarray(inputs["b1"], dtype=np.float32)
    gamma = np.asarray(inputs["gamma"], dtype=np.float32)
    beta = np.asarray(inputs["beta"], dtype=np.float32)
    w2 = np.asarray(inputs["w2"], dtype=np.float32)
    b2 = np.asarray(inputs["b2"], dtype=np.float32)
    eps_gin = float(np.asarray(inputs["eps_gin"]))

    sharded, in_names, out_names, out_avals = _get_runner()

    xb = np.ascontiguousarray(x[:, :, :, 0])                     # [B, C, N]
    vecs = np.stack(
        [b1, gamma, beta, b2, np.full(O, 1.0 + eps_gin, np.float32)], axis=1
    ).astype(np.float32)                                         # [64, 5]
    ones_row = np.ones((1, N), np.float32)
    ones_col = np.ones((C, 1), np.float32)

    ident = np.eye(C, dtype=ml_dtypes.bfloat16)
    per_core = {
        "xb": xb,
        "ident64": np.broadcast_to(ident, (N_CORES,) + ident.shape),
        "w1": np.broadcast_to(w1, (N_CORES,) + w1.shape),
        "w2": np.broadcast_to(w2, (N_CORES,) + w2.shape),
        "vecs": np.broadcast_to(vecs, (N_CORES,) + vecs.shape),
        "ones_row": np.broadcast_to(ones_row, (N_CORES,) + ones_row.shape),
        "ones_col": np.broadcast_to(ones_col, (N_CORES,) + ones_col.shape),
    }
    # shard_map in_specs=P("core") take global arrays concatenated on axis 0
    concat_in = [
        np.ascontiguousarray(per_core[name]).reshape(
            (N_CORES * per_core[name].shape[1],) + per_core[name].shape[2:]
        )
        for name in in_names
    ]
    out_arrs = sharded(*concat_in)
    yi = out_names.index("y")
    y = np.asarray(out_arrs[yi]).reshape(N_CORES, O, N)
    return y[..., None].astype(np.float32)



# revision 15
# speedup vs baseline: 1.6353x; 1.6353x over previous
"""DyGraphGIN2d Trainium kernel: kNN graph (k=16) + GIN aggregation + MLP/BN/GELU.

Sharding: data-parallel over batch B=8 across 8 NeuronCores (one batch
element per core). BatchNorm uses training-mode batch statistics over ALL
B*N rows, so per-core partial sums go through one in-kernel AllReduce.

Algorithm per core (N=4096 points, C=64 dims), single-matmul phases:
  The ranking metric s[n,m] = <x_hi_n, x_hi_m> + q_m with q = -|x|^2/2
  carried EXACTLY as two f32r rank-1 rows (q_hi + q_lo, an exact hi/lo
  split), all folded into ONE K=66 f32r matmul per 128x512 block (extra
  contraction rows are free: matmul cost is output-columns only).
  Phase 1 (tau): per 128-row stripe, 8 such matmuls + DVE top-8 per
  512-chunk + max/match_replace/max give the 16th-largest s per row;
  tau rides back into the XL operand as two more exact f32r rank-1 rows
  (-tau_hi, -tau_lo).
  Phase 2 (mask+aggregate): v' = s^T - tau is recomputed transposed by the
  mirrored K=68 matmul -- the first 66 product terms are bitwise identical
  to phase 1 (commuted multiplies, same PSUM order), so selection is
  bit-consistent; the 1e-5 guard inside tau makes v' > 0 strict for the
  16 selected neighbors. Masks {0,1} are made OFF the DVE: ACT computes
  sigmoid(4e6 * v') (saturates to exactly 1.0/0.0 in f32r) and Pool(GPSIMD)
  computes is_gt(v', 0), alternating per block. aggr[c,n] accumulates in
  PSUM via single-pass f32r matmuls with x_hi^T (PE-transposed).
  The phase-1 stripes (DVE-bound) and phase-2 blocks (PE-bound) are
  software-pipelined chunk-by-chunk so PE/DVE/ACT/Pool all stay busy.
  Tail: h = (1+eps)x + aggr (Pool); h1 = W1^T h + b1 (f32r PE + ACT bias
  with accum_out BN sums); BN stats AllReduce; fused BN+erf-GELU on ACT;
  out = W2^T hg + b2.

The jitted 8-core shard_map executable is cached across kernel() calls.
"""

import numpy as np

import concourse.bacc as bacc
import concourse.mybir as mybir
from concourse.tile import TileContext

F32 = mybir.dt.float32
F32R = mybir.dt.float32r
AF = mybir.ActivationFunctionType
ALU = mybir.AluOpType

B, C, N, O = 8, 64, 4096, 64
K_NN = 16
N_CORES = 8
NT = N // 128          # 32 row stripes
NCH = 8                # 512-wide column chunks
BN_EPS = 1e-5
BN_COUNT = float(B * N)
TAU_GUARD = 1e-5
SIG_SCALE = 4e6

_cache = {}


def _build():
    nc = bacc.Bacc("TRN2", target_bir_lowering=False)

    xb_d = nc.dram_tensor("xb", [C, N], F32, kind="ExternalInput")
    w1_d = nc.dram_tensor("w1", [C, O], F32, kind="ExternalInput")
    w2_d = nc.dram_tensor("w2", [O, O], F32, kind="ExternalInput")
    vecs_d = nc.dram_tensor("vecs", [O, 5], F32, kind="ExternalInput")  # b1,gamma,beta,b2,eps1
    ones2_d = nc.dram_tensor("ones2", [2, N], F32R, kind="ExternalInput")
    onesc_d = nc.dram_tensor("ones_col", [C, 1], F32, kind="ExternalInput")
    identr_d = nc.dram_tensor("identr", [C, C], F32R, kind="ExternalInput")
    y_d = nc.dram_tensor("y", [O, N], F32, kind="ExternalOutput")
    tau_scr = nc.dram_tensor("tau_scr", [N, 2], F32R)  # internal scratch

    with TileContext(nc) as tc:
        with tc.tile_pool(name="big", bufs=1) as big, \
             tc.tile_pool(name="work", bufs=1) as work, \
             tc.tile_pool(name="dram", bufs=1, space="DRAM") as dpool:

            # ---------------- prologue: operands ----------------
            vecs_sb = work.tile([O, 5], F32)
            w1_sb = work.tile([C, O], F32)
            w2_sb = work.tile([O, O], F32)
            identr = work.tile([C, C], F32R)
            onesc = work.tile([C, 1], F32)
            nc.sync.dma_start(vecs_sb[:, :], vecs_d[:, :])
            nc.sync.dma_start(w1_sb[:, :], w1_d[:, :])
            nc.sync.dma_start(w2_sb[:, :], w2_d[:, :])
            nc.sync.dma_start(identr[:, :], identr_d[:, :])
            nc.sync.dma_start(onesc[:, :], onesc_d[:, :])
            w1r = work.tile([C, O], F32R)
            w2r = work.tile([O, O], F32R)
            nc.scalar.activation(w1r[:, :], w1_sb[:, :], AF.Copy)
            nc.scalar.activation(w2r[:, :], w2_sb[:, :], AF.Copy)

            xbc = [big.tile([C, 512], F32, name=f"xbc{i}") for i in range(NCH)]
            # XLc: p1 lhsT [x_hi; 1; 1] (never written after prologue).
            # XL2c: p2 rhs [x_hi; 1; 1; -tau_hi; -tau_lo] -- a separate copy so
            # the per-stripe tau-row DMAs don't put whole-tile false deps on
            # the p1 stripes still reading XLc.
            XLc = [big.tile([128, 512], F32R, name=f"XLc{i}") for i in range(NCH)]
            XL2c = [big.tile([128, 512], F32R, name=f"XL2c{i}") for i in range(NCH)]
            XRc = [big.tile([128, 512], F32R, name=f"XRc{i}") for i in range(NCH)]
            xt_sb = big.tile([128, NT * C], F32R)

            xsq = work.tile([C, 512], F32, tag="xsq", bufs=2)
            qh = work.tile([1, 512], F32R, tag="qh", bufs=2)
            qt = work.tile([1, 512], F32, tag="qt", bufs=2)
            with tc.tile_pool(name="ps_sq", bufs=2, space="PSUM") as ps_sq:
                for c in range(NCH):
                    sl = slice(c * 512, (c + 1) * 512)
                    nc.sync.dma_start(xbc[c][:, :], xb_d[:, sl])
                    nc.scalar.activation(XLc[c][:C, :], xbc[c][:, :], AF.Copy)
                    nc.sync.dma_start(XLc[c][C : C + 2, :], ones2_d[:, sl])
                    nc.sync.dma_start(XRc[c][:C, :], XLc[c][:C, :])
                    nc.sync.dma_start(XRc[c][C + 2 : C + 4, :], ones2_d[:, sl])
                    nc.sync.dma_start(XL2c[c][: C + 2, :], XLc[c][: C + 2, :])
                    # q = -|x_m|^2/2 exactly as f32r hi+lo rank-1 rows
                    nc.gpsimd.tensor_tensor(out=xsq[:, :], in0=xbc[c][:, :],
                                            in1=xbc[c][:, :], op=ALU.mult)
                    sq_ps = ps_sq.tile([1, 512], F32, tag="sq_ps")
                    nc.tensor.matmul(sq_ps[:, :], onesc[:C, :], xsq[:, :],
                                     start=True, stop=True)
                    nc.scalar.activation(XRc[c][C : C + 1, :], sq_ps[:, :], AF.Copy,
                                         scale=-0.5)
                    nc.scalar.activation(qh[:, :], sq_ps[:, :], AF.Copy, scale=-0.5)
                    nc.vector.tensor_scalar(out=qt[:, :], in0=sq_ps[:, :],
                                            scalar1=-0.5, scalar2=None, op0=ALU.mult)
                    nc.vector.tensor_tensor(out=qt[:, :], in0=qt[:, :],
                                            in1=qh.bitcast(F32)[:, :], op=ALU.subtract)
                    # ACT can only write at partition base 0/64; q_lo (row 65)
                    # goes through a partition-0 staging tile + DMA.
                    ql_t = work.tile([1, 512], F32R, tag="ql", bufs=2,
                                     name=f"ql_{c}")
                    nc.scalar.activation(ql_t[:, :], qt[:, :], AF.Copy)
                    nc.sync.dma_start(XRc[c][C + 1 : C + 2, :], ql_t[:, :])

            # x_hi^T chunks for the aggregation matmuls (PE transpose)
            with tc.tile_pool(name="ps_tp", bufs=2, space="PSUM") as ps_tp:
                for j in range(NT):
                    tp = ps_tp.tile([128, C], F32R, tag="tp_ps")
                    nc.tensor.transpose(tp[:, :],
                                        XLc[j // 4][:C, (j % 4) * 128 : (j % 4 + 1) * 128],
                                        identr[:, :])
                    nc.scalar.activation(xt_sb[:, j * C : (j + 1) * C],
                                         tp[:, :], AF.Copy)

            # ---------------- main pipelined loop ----------------
            ps_s_cm = tc.tile_pool(name="ps_s", bufs=3, space="PSUM")
            ps_v_cm = tc.tile_pool(name="ps_v", bufs=3, space="PSUM")
            ps_a_cm = tc.tile_pool(name="ps_a", bufs=2, space="PSUM")
            ps_s = ps_s_cm.__enter__()
            ps_v = ps_v_cm.__enter__()
            ps_a = ps_a_cm.__enter__()
            _cms = [ps_s_cm, ps_v_cm, ps_a_cm]

            cand = work.tile([128, 64], F32, tag="cand", bufs=2)
            t8a = work.tile([128, 8], F32, tag="t8a", bufs=2)
            t8b = work.tile([128, 8], F32, tag="t8b", bufs=2)
            ntf = work.tile([128, 1], F32, tag="ntf", bufs=2)
            ntau2 = work.tile([128, 2], F32R, tag="ntau2", bufs=2)
            mask = work.tile([128, 512], F32R, tag="mask", bufs=6)
            hc = work.tile([C, 512], F32R, tag="hc", bufs=3)
            h1c = [big.tile([O, 512], F32, name=f"h1c{i}") for i in range(NCH)]
            bnsum = work.tile([O, NCH], F32)
            bnsq = work.tile([O, NCH], F32)
            sqscr = work.tile([O, 512], F32, tag="sqscr", bufs=2)
            eps1 = vecs_sb[:, 4:5]

            aggr_tiles = {}

            def p1_mm(s, c8, cand_t):
                jt, jo = s // 4, (s % 4) * 128
                s_ps = ps_s.tile([128, 512], F32, tag="s_ps", name=f"s_{s}_{c8}")
                nc.tensor.matmul(s_ps[:, :], XLc[jt][: C + 2, jo : jo + 128],
                                 XRc[c8][: C + 2, :], start=True, stop=True)
                nc.vector.max(out=cand_t[:, c8 * 8 : (c8 + 1) * 8], in_=s_ps[:, :])

            def p1_tail(s, cand_t):
                jt, jo = s // 4, (s % 4) * 128
                t8a_t = work.tile([128, 8], F32, tag="t8a", bufs=2, name=f"t8a_{s}")
                t8b_t = work.tile([128, 8], F32, tag="t8b", bufs=2, name=f"t8b_{s}")
                ntf_t = work.tile([128, 1], F32, tag="ntf", bufs=2, name=f"ntf_{s}")
                nt2_t = work.tile([128, 2], F32R, tag="ntau2", bufs=2, name=f"nt2_{s}")
                nc.vector.max(out=t8a_t[:, :], in_=cand_t[:, :])
                nc.vector.match_replace(out=cand_t[:, :], in_to_replace=t8a_t[:, :],
                                        in_values=cand_t[:, :], imm_value=-1e30)
                nc.vector.max(out=t8b_t[:, :], in_=cand_t[:, :])
                # -tau = -(t16 - guard) = guard - t16, split exactly hi+lo
                nc.gpsimd.tensor_scalar(out=ntf_t[:, :], in0=t8b_t[:, 7:8],
                                        scalar1=-1.0, scalar2=TAU_GUARD,
                                        op0=ALU.mult, op1=ALU.add)
                nc.scalar.activation(nt2_t[:, 0:1], ntf_t[:, :], AF.Copy)
                nc.gpsimd.tensor_tensor(out=nt2_t.bitcast(F32)[:, 1:2],
                                        in0=ntf_t[:, :],
                                        in1=nt2_t.bitcast(F32)[:, 0:1],
                                        op=ALU.subtract)
                # SBUF->SBUF DMA cannot transpose partition->free; bounce
                # the per-stripe [128,2] tau pair through flat DRAM.
                nc.sync.dma_start(tau_scr[s * 128 : (s + 1) * 128, :], nt2_t[:, 0:2])
                nc.sync.dma_start(
                    XL2c[jt][C + 2 : C + 4, jo : jo + 128],
                    tau_scr[s * 128 : (s + 1) * 128, 0:2].rearrange("p two -> two p"))

            def p2_v(c, j):
                jt, jo = j // 4, (j % 4) * 128
                v_ps = ps_v.tile([128, 512], F32, tag="v_ps", name=f"v_{c}_{j}")
                nc.tensor.matmul(v_ps[:, :], XRc[jt][: C + 4, jo : jo + 128],
                                 XL2c[c][: C + 4, :], start=True, stop=True)
                m = work.tile([128, 512], F32R, tag="mask", bufs=6, name=f"m_{c}_{j}")
                nc.scalar.activation(m[:, :], v_ps[:, :], AF.Sigmoid,
                                     scale=SIG_SCALE)
                return m

            def p2_aggr(c, j, m):
                nc.tensor.matmul(aggr_tiles[c][:, :],
                                 xt_sb[:, j * C : (j + 1) * C],
                                 m[:, :],
                                 start=(j == 0), stop=(j == NT - 1))

            def finish_mlp(c):
                sl = slice(c * 512, (c + 1) * 512)
                xe_t = work.tile([C, 512], F32, tag="xe", bufs=2, name=f"xe_{c}")
                nc.gpsimd.tensor_scalar(out=xe_t[:, :], in0=xbc[c][:, :],
                                        scalar1=eps1, scalar2=None, op0=ALU.mult)
                h_t = work.tile([C, 512], F32R, tag="hc", bufs=3, name=f"h_{c}")
                nc.vector.tensor_tensor(out=h_t[:, :], in0=xe_t[:, :],
                                        in1=aggr_tiles[c][:, :], op=ALU.add)
                h1_ps = ps_v.tile([O, 512], F32, tag="v_ps", name=f"h1ps_{c}")
                nc.tensor.matmul(h1_ps[:, :], w1r[:, :], h_t[:, :],
                                 start=True, stop=True)
                nc.scalar.activation(h1c[c][:, :], h1_ps[:, :], AF.Identity,
                                     bias=vecs_sb[:, 0:1],
                                     accum_out=bnsum[:, c : c + 1])
                sq_t = work.tile([O, 512], F32, tag="sqscr", bufs=2, name=f"sq_{c}")
                nc.scalar.activation(sq_t[:, :], h1c[c][:, :], AF.Square,
                                     accum_out=bnsq[:, c : c + 1])

            # software pipeline: iteration it runs phase-1 stripes of chunk
            # it and phase-2 of chunk it-1, interleaved 1:1 on the PE stream.
            for it in range(NCH + 1):
                c1 = it if it < NCH else None
                c2 = it - 1 if it >= 1 else None
                if c2 is not None:
                    aggr_tiles[c2] = ps_a.tile([O, 512], F32, tag="aggr",
                                               name=f"aggr_{c2}")
                cands = {}
                masks = {}
                for k in range(NT):
                    if c1 is not None:
                        s = 4 * c1 + k // 8
                        if k % 8 == 0:
                            cands[s] = work.tile([128, 64], F32, tag="cand",
                                                 bufs=2, name=f"cand_{s}")
                        p1_mm(s, k % 8, cands[s])
                    if c2 is not None:
                        masks[k] = p2_v(c2, k)
                        if k >= 2:
                            p2_aggr(c2, k - 2, masks.pop(k - 2))
                    if c1 is not None and k % 8 == 7:
                        p1_tail(4 * c1 + k // 8, cands[4 * c1 + k // 8])
                if c2 is not None:
                    p2_aggr(c2, NT - 2, masks.pop(NT - 2))
                    p2_aggr(c2, NT - 1, masks.pop(NT - 1))
                    finish_mlp(c2)

            # ---------------- BN combine + AllReduce + GELU + W2 ---------
            stats = work.tile([O, 2], F32)
            nc.vector.reduce_sum(stats[:, 0:1], bnsum[:, :], axis=mybir.AxisListType.X)
            nc.vector.reduce_sum(stats[:, 1:2], bnsq[:, :], axis=mybir.AxisListType.X)

            cc_in = dpool.tile([O, 2], F32)
            cc_out = dpool.tile([O, 2], F32, addr_space="Shared")
            nc.sync.dma_start(cc_in[:, :], stats[:, :])
            nc.gpsimd.collective_compute(
                "AllReduce", ALU.add,
                ins=[cc_in[:, :]],
                outs=[cc_out[:, :]],
                replica_groups=[list(range(N_CORES))],
            )
            gstats = work.tile([O, 2], F32)
            nc.sync.dma_start(gstats[:, :], cc_out[:, :])

            mean = work.tile([O, 1], F32)
            var = work.tile([O, 1], F32)
            scale = work.tile([O, 1], F32)
            shift = work.tile([O, 1], F32)
            tmp = work.tile([O, 1], F32)
            nc.vector.tensor_scalar(out=mean[:, :], in0=gstats[:, 0:1],
                                    scalar1=1.0 / BN_COUNT, scalar2=None, op0=ALU.mult)
            nc.vector.tensor_scalar(out=var[:, :], in0=gstats[:, 1:2],
                                    scalar1=1.0 / BN_COUNT, scalar2=None, op0=ALU.mult)
            nc.vector.tensor_tensor(out=tmp[:, :], in0=mean[:, :], in1=mean[:, :],
                                    op=ALU.mult)
            nc.vector.tensor_tensor(out=var[:, :], in0=var[:, :], in1=tmp[:, :],
                                    op=ALU.subtract)
            nc.vector.tensor_scalar(out=var[:, :], in0=var[:, :], scalar1=BN_EPS,
                                    scalar2=None, op0=ALU.add)
            nc.scalar.activation(tmp[:, :], var[:, :], AF.Sqrt)
            nc.vector.reciprocal(out=tmp[:, :], in_=tmp[:, :])
            nc.vector.tensor_tensor(out=scale[:, :], in0=vecs_sb[:, 1:2],
                                    in1=tmp[:, :], op=ALU.mult)
            nc.vector.tensor_tensor(out=tmp[:, :], in0=mean[:, :], in1=scale[:, :],
                                    op=ALU.mult)
            nc.vector.tensor_tensor(out=shift[:, :], in0=vecs_sb[:, 2:3],
                                    in1=tmp[:, :], op=ALU.subtract)

            hg = work.tile([O, 512], F32R, tag="hg", bufs=3)
            y_sb = work.tile([O, 512], F32, tag="ysb", bufs=3)
            for c in range(NCH):
                sl = slice(c * 512, (c + 1) * 512)
                hg_t = work.tile([O, 512], F32R, tag="hg", bufs=3, name=f"hg_{c}")
                nc.scalar.activation(hg_t[:, :], h1c[c][:, :], AF.Gelu,
                                     scale=scale[:, :], bias=shift[:, :])
                o_ps = ps_v.tile([O, 512], F32, tag="v_ps", name=f"ops_{c}")
                nc.tensor.matmul(o_ps[:, :], w2r[:, :], hg_t[:, :],
                                 start=True, stop=True)
                y_t = work.tile([O, 512], F32, tag="ysb", bufs=3, name=f"y_{c}")
                nc.scalar.activation(y_t[:, :], o_ps[:, :], AF.Identity,
                                     bias=vecs_sb[:, 3:4])
                nc.sync.dma_start(y_d[:, sl], y_t[:, :])

            for cm in reversed(_cms):
                cm.__exit__(None, None, None)

    if not nc.is_finalized():
        nc.finalize()
    return nc


def _get_runner():
    """Build the Bass module once and cache a jitted 8-core executable."""
    if "runner" in _cache:
        return _cache["runner"]

    import jax
    import concourse.mybir as mb
    from jax.sharding import Mesh, PartitionSpec
    from jax.experimental.shard_map import shard_map
    from concourse import bass2jax

    nc = _build()
    bass2jax.install_neuronx_cc_hook()

    partition_name = nc.partition_id_tensor.name if nc.partition_id_tensor else None
    in_names = []
    out_names = []
    out_avals = []
    for alloc in nc.m.functions[0].allocations:
        if not isinstance(alloc, mb.MemoryLocationSet):
            continue
        name = alloc.memorylocations[0].name
        if alloc.kind == "ExternalInput":
            if name != partition_name:
                in_names.append(name)
        elif alloc.kind == "ExternalOutput":
            out_names.append(name)
            out_avals.append(
                jax.core.ShapedArray(tuple(alloc.tensor_shape), mb.dt.np(alloc.dtype))
            )
    n_params = len(in_names)
    all_in_names = list(in_names)
    if partition_name is not None:
        all_in_names = all_in_names + [partition_name]

    def _body(*args):
        operands = list(args)
        if partition_name is not None:
            operands.append(bass2jax.partition_id_tensor())
        outs = bass2jax._bass_exec_p.bind(
            *operands,
            out_avals=tuple(out_avals),
            in_names=tuple(all_in_names),
            out_names=tuple(out_names),
            lowering_input_output_aliases=(),
            sim_require_finite=True,
            sim_require_nnan=True,
            nc=nc,
        )
        return tuple(outs)

    devices = jax.devices()[:N_CORES]
    assert len(devices) == N_CORES, f"need {N_CORES} devices, have {len(jax.devices())}"
    mesh = Mesh(np.asarray(devices), ("core",))
    n_outs = len(out_names)
    sharded = jax.jit(
        shard_map(
            _body,
            mesh=mesh,
            in_specs=(PartitionSpec("core"),) * n_params,
            out_specs=(PartitionSpec("core"),) * n_outs,
            check_rep=False,
        ),
        keep_unused=True,
    )
    _cache["runner"] = (sharded, in_names, out_names, out_avals)
    return _cache["runner"]


def kernel(**inputs) -> np.ndarray:
    x = np.asarray(inputs["x"], dtype=np.float32)
    assert x.shape == (B, C, N, 1), x.shape
    k = int(np.asarray(inputs.get("k", K_NN)))
    assert k == K_NN, f"kernel compiled for k={K_NN}, got {k}"
    w1 = np.asarray(inputs["w1"], dtype=np.float32)
    b1 = np.asarray(inputs["b1"], dtype=np.float32)
    gamma = np.asarray(inputs["gamma"], dtype=np.float32)
    beta = np.asarray(inputs["beta"], dtype=np.float32)
    w2 = np.asarray(inputs["w2"], dtype=np.float32)
    b2 = np.asarray(inputs["b2"], dtype=np.float32)
    eps_gin = float(np.asarray(inputs["eps_gin"]))

    sharded, in_names, out_names, out_avals = _get_runner()

    xb = np.ascontiguousarray(x[:, :, :, 0])                     # [B, C, N]
    vecs = np.stack(
        [b1, gamma, beta, b2, np.full(O, 1.0 + eps_gin, np.float32)], axis=1
    ).astype(np.float32)                                         # [64, 5]
    ones2 = np.ones((2, N), np.float32)
    ones_col = np.ones((C, 1), np.float32)
    identr = np.eye(C, dtype=np.float32)

    per_core = {
        "xb": xb,
        "w1": np.broadcast_to(w1, (N_CORES,) + w1.shape),
        "w2": np.broadcast_to(w2, (N_CORES,) + w2.shape),
        "vecs": np.broadcast_to(vecs, (N_CORES,) + vecs.shape),
        "ones2": np.broadcast_to(ones2, (N_CORES,) + ones2.shape),
        "ones_col": np.broadcast_to(ones_col, (N_CORES,) + ones_col.shape),
        "identr": np.broadcast_to(identr, (N_CORES,) + identr.shape),
    }
    concat_in = [
        np.ascontiguousarray(per_core[name]).reshape(
            (N_CORES * per_core[name].shape[1],) + per_core[name].shape[2:]
        )
        for name in in_names
    ]
    out_arrs = sharded(*concat_in)
    yi = out_names.index("y")
    y = np.asarray(out_arrs[yi]).reshape(N_CORES, O, N)
    return y[..., None].astype(np.float32)


# revision 19
# speedup vs baseline: 1.7285x; 1.0570x over previous
"""DyGraphGIN2d Trainium kernel: kNN graph (k=16) + GIN aggregation + MLP/BN/GELU.

Sharding: data-parallel over batch B=8 across 8 NeuronCores (one batch
element per core). BatchNorm uses training-mode batch statistics over ALL
B*N rows, so per-core partial sums go through one in-kernel AllReduce.

Algorithm per core (N=4096 points, C=64 dims), single-matmul phases:
  The ranking metric s[n,m] = <x_hi_n, x_hi_m> + q_m with q = -|x|^2/2
  carried EXACTLY as two f32r rank-1 rows (q_hi + q_lo, an exact hi/lo
  split), all folded into ONE K=66 f32r matmul per 128x512 block (extra
  contraction rows are free: matmul cost is output-columns only).
  Phase 1 (tau): per 128-row stripe, 8 such matmuls + DVE top-8 per
  512-chunk + max/match_replace/max give the 16th-largest s per row;
  tau rides back into the XL operand as two more exact f32r rank-1 rows
  (-tau_hi, -tau_lo).
  Phase 2 (mask+aggregate): v' = s^T - tau is recomputed transposed by the
  mirrored K=68 matmul -- the first 66 product terms are bitwise identical
  to phase 1 (commuted multiplies, same PSUM order), so selection is
  bit-consistent; the 1e-5 guard inside tau makes v' > 0 strict for the
  16 selected neighbors. Masks {0,1} are made OFF the DVE: ACT computes
  sigmoid(4e6 * v') (saturates to exactly 1.0/0.0 in f32r) and Pool(GPSIMD)
  computes is_gt(v', 0), alternating per block. aggr[c,n] accumulates in
  PSUM via single-pass f32r matmuls with x_hi^T (PE-transposed).
  The phase-1 stripes (DVE-bound) and phase-2 blocks (PE-bound) are
  software-pipelined chunk-by-chunk so PE/DVE/ACT/Pool all stay busy.
  Tail: h = (1+eps)x + aggr (Pool); h1 = W1^T h + b1 (f32r PE + ACT bias
  with accum_out BN sums); BN stats AllReduce; fused BN+erf-GELU on ACT;
  out = W2^T hg + b2.

The jitted 8-core shard_map executable is cached across kernel() calls.
"""

import numpy as np

import concourse.bacc as bacc
import concourse.mybir as mybir
from concourse.tile import TileContext

F32 = mybir.dt.float32
F32R = mybir.dt.float32r
AF = mybir.ActivationFunctionType
ALU = mybir.AluOpType

B, C, N, O = 8, 64, 4096, 64
K_NN = 16
N_CORES = 8
NT = N // 128          # 32 row stripes
NCH = 8                # 512-wide column chunks
BN_EPS = 1e-5
BN_COUNT = float(B * N)
TAU_GUARD = 1e-5
SIG_SCALE = 4e6

_cache = {}


def _build():
    nc = bacc.Bacc("TRN2", target_bir_lowering=False)

    xb_d = nc.dram_tensor("xb", [C, N], F32, kind="ExternalInput")
    w1_d = nc.dram_tensor("w1", [C, O], F32, kind="ExternalInput")
    w2_d = nc.dram_tensor("w2", [O, O], F32, kind="ExternalInput")
    vecs_d = nc.dram_tensor("vecs", [O, 5], F32, kind="ExternalInput")  # b1,gamma,beta,b2,eps1
    ones2_d = nc.dram_tensor("ones2", [2, N], F32R, kind="ExternalInput")
    onesc_d = nc.dram_tensor("ones_col", [C, 1], F32, kind="ExternalInput")
    identr_d = nc.dram_tensor("identr", [C, C], F32R, kind="ExternalInput")
    y_d = nc.dram_tensor("y", [O, N], F32, kind="ExternalOutput")
    tau_scr = nc.dram_tensor("tau_scr", [N, 2], F32R)  # internal scratch

    with TileContext(nc) as tc:
        with tc.tile_pool(name="big", bufs=1) as big, \
             tc.tile_pool(name="work", bufs=1) as work, \
             tc.tile_pool(name="dram", bufs=1, space="DRAM") as dpool:

            # ---------------- prologue: operands ----------------
            vecs_sb = work.tile([O, 5], F32)
            w1_sb = work.tile([C, O], F32)
            w2_sb = work.tile([O, O], F32)
            identr = work.tile([C, C], F32R)
            onesc = work.tile([C, 1], F32)
            nc.sync.dma_start(vecs_sb[:, :], vecs_d[:, :])
            nc.sync.dma_start(w1_sb[:, :], w1_d[:, :])
            nc.sync.dma_start(w2_sb[:, :], w2_d[:, :])
            nc.sync.dma_start(identr[:, :], identr_d[:, :])
            nc.sync.dma_start(onesc[:, :], onesc_d[:, :])
            w1r = work.tile([C, O], F32R)
            w2r = work.tile([O, O], F32R)
            nc.scalar.activation(w1r[:, :], w1_sb[:, :], AF.Copy)
            nc.scalar.activation(w2r[:, :], w2_sb[:, :], AF.Copy)

            xbc = [big.tile([C, 512], F32, name=f"xbc{i}") for i in range(NCH)]
            # XLc: p1 lhsT rows [x_hi; 1; 1], p2 rhs rows [x_hi; 1; 1;
            # -tau_hi; -tau_lo].  The tau rows are DMA-written only after all
            # p1 stripes of the chunk have been emitted, so no false
            # whole-tile WAR stalls arise.
            XLc = [big.tile([128, 512], F32R, name=f"XLc{i}") for i in range(NCH)]
            XRc = [big.tile([128, 512], F32R, name=f"XRc{i}") for i in range(NCH)]
            xt_sb = big.tile([128, NT * C], F32R)

            xsq = work.tile([C, 512], F32, tag="xsq", bufs=2)
            qh = work.tile([1, 512], F32R, tag="qh", bufs=2)
            qt = work.tile([1, 512], F32, tag="qt", bufs=2)
            with tc.tile_pool(name="ps_sq", bufs=2, space="PSUM") as ps_sq:
                for c in range(NCH):
                    sl = slice(c * 512, (c + 1) * 512)
                    nc.sync.dma_start(xbc[c][:, :], xb_d[:, sl])
                    nc.scalar.activation(XLc[c][:C, :], xbc[c][:, :], AF.Copy)
                    nc.sync.dma_start(XLc[c][C : C + 2, :], ones2_d[:, sl])
                    nc.gpsimd.tensor_copy(XRc[c][:C, :], XLc[c][:C, :])
                    nc.sync.dma_start(XRc[c][C + 2 : C + 4, :], ones2_d[:, sl])
                    # q = -|x_m|^2/2 exactly as f32r hi+lo rank-1 rows
                    nc.gpsimd.tensor_tensor(out=xsq[:, :], in0=xbc[c][:, :],
                                            in1=xbc[c][:, :], op=ALU.mult)
                    sq_ps = ps_sq.tile([1, 512], F32, tag="sq_ps")
                    nc.tensor.matmul(sq_ps[:, :], onesc[:C, :], xsq[:, :],
                                     start=True, stop=True)
                    nc.scalar.activation(XRc[c][C : C + 1, :], sq_ps[:, :], AF.Copy,
                                         scale=-0.5)
                    nc.scalar.activation(qh[:, :], sq_ps[:, :], AF.Copy, scale=-0.5)
                    nc.vector.tensor_scalar(out=qt[:, :], in0=sq_ps[:, :],
                                            scalar1=-0.5, scalar2=None, op0=ALU.mult)
                    nc.vector.tensor_tensor(out=qt[:, :], in0=qt[:, :],
                                            in1=qh.bitcast(F32)[:, :], op=ALU.subtract)
                    # ACT can only write at partition base 0/64; q_lo (row 65)
                    # goes through a partition-0 staging tile + DMA.
                    ql_t = work.tile([1, 512], F32R, tag="ql", bufs=2,
                                     name=f"ql_{c}")
                    nc.scalar.activation(ql_t[:, :], qt[:, :], AF.Copy)
                    nc.sync.dma_start(XRc[c][C + 1 : C + 2, :], ql_t[:, :])

            # x_hi^T chunks for the aggregation matmuls (PE transpose)
            with tc.tile_pool(name="ps_tp", bufs=2, space="PSUM") as ps_tp:
                for j in range(NT):
                    tp = ps_tp.tile([128, C], F32R, tag="tp_ps")
                    nc.tensor.transpose(tp[:, :],
                                        XLc[j // 4][:C, (j % 4) * 128 : (j % 4 + 1) * 128],
                                        identr[:, :])
                    nc.scalar.activation(xt_sb[:, j * C : (j + 1) * C],
                                         tp[:, :], AF.Copy)

            # ---------------- main pipelined loop ----------------
            ps_s_cm = tc.tile_pool(name="ps_s", bufs=3, space="PSUM")
            ps_v_cm = tc.tile_pool(name="ps_v", bufs=3, space="PSUM")
            ps_a_cm = tc.tile_pool(name="ps_a", bufs=2, space="PSUM")
            ps_s = ps_s_cm.__enter__()
            ps_v = ps_v_cm.__enter__()
            ps_a = ps_a_cm.__enter__()
            _cms = [ps_s_cm, ps_v_cm, ps_a_cm]

            cand = work.tile([128, 64], F32, tag="cand", bufs=5)
            t8a = work.tile([128, 8], F32, tag="t8a", bufs=4)
            t8b = work.tile([128, 8], F32, tag="t8b", bufs=4)
            ntf = work.tile([128, 1], F32, tag="ntf", bufs=4)
            ntau2 = work.tile([128, 2], F32R, tag="ntau2", bufs=4)
            mask = work.tile([128, 512], F32R, tag="mask", bufs=6)
            hc = work.tile([C, 512], F32R, tag="hc", bufs=3)
            h1c = [big.tile([O, 512], F32, name=f"h1c{i}") for i in range(NCH)]
            bnsum = work.tile([O, NCH], F32)
            bnsq = work.tile([O, NCH], F32)
            sqscr = work.tile([O, 512], F32, tag="sqscr", bufs=2)
            eps1 = vecs_sb[:, 4:5]

            aggr_tiles = {}

            def p1_mm(s, c8, cand_t):
                jt, jo = s // 4, (s % 4) * 128
                s_ps = ps_s.tile([128, 512], F32, tag="s_ps", name=f"s_{s}_{c8}")
                nc.tensor.matmul(s_ps[:, :], XLc[jt][: C + 2, jo : jo + 128],
                                 XRc[c8][: C + 2, :], start=True, stop=True)
                nc.vector.max(out=cand_t[:, c8 * 8 : (c8 + 1) * 8], in_=s_ps[:, :])

            def p1_tail(s, cand_t):
                jt, jo = s // 4, (s % 4) * 128
                t8a_t = work.tile([128, 8], F32, tag="t8a", bufs=4, name=f"t8a_{s}")
                t8b_t = work.tile([128, 8], F32, tag="t8b", bufs=4, name=f"t8b_{s}")
                ntf_t = work.tile([128, 1], F32, tag="ntf", bufs=4, name=f"ntf_{s}")
                nt2_t = work.tile([128, 2], F32R, tag="ntau2", bufs=4, name=f"nt2_{s}")
                nc.vector.max(out=t8a_t[:, :], in_=cand_t[:, :])
                nc.vector.match_replace(out=cand_t[:, :], in_to_replace=t8a_t[:, :],
                                        in_values=cand_t[:, :], imm_value=-1e30)
                nc.vector.max(out=t8b_t[:, :], in_=cand_t[:, :])
                # -tau = -(t16 - guard) = guard - t16, split exactly hi+lo
                nc.gpsimd.tensor_scalar(out=ntf_t[:, :], in0=t8b_t[:, 7:8],
                                        scalar1=-1.0, scalar2=TAU_GUARD,
                                        op0=ALU.mult, op1=ALU.add)
                nc.vector.tensor_copy(nt2_t[:, 0:1], ntf_t[:, :])
                nc.gpsimd.tensor_tensor(out=nt2_t.bitcast(F32)[:, 1:2],
                                        in0=ntf_t[:, :],
                                        in1=nt2_t.bitcast(F32)[:, 0:1],
                                        op=ALU.subtract)
                # SBUF->SBUF DMA cannot transpose partition->free; bounce
                # the per-stripe [128,2] tau pair through flat DRAM.  The
                # chunk-wide readback into XLc happens in tau_readback().
                nc.sync.dma_start(tau_scr[s * 128 : (s + 1) * 128, :], nt2_t[:, 0:2])

            def p2_v(c, j):
                jt, jo = j // 4, (j % 4) * 128
                v_ps = ps_v.tile([128, 512], F32, tag="v_ps", name=f"v_{c}_{j}")
                nc.tensor.matmul(v_ps[:, :], XRc[jt][: C + 4, jo : jo + 128],
                                 XLc[c][: C + 4, :], start=True, stop=True)
                m = work.tile([128, 512], F32R, tag="mask", bufs=6, name=f"m_{c}_{j}")
                if c == NCH - 1:
                    nc.vector.tensor_scalar(out=m[:, :], in0=v_ps[:, :],
                                            scalar1=0.0, scalar2=None,
                                            op0=ALU.is_gt)
                else:
                    nc.scalar.activation(m[:, :], v_ps[:, :], AF.Sigmoid,
                                         scale=SIG_SCALE)
                return m

            def p2_aggr(c, j, m):
                nc.tensor.matmul(aggr_tiles[c][:, :],
                                 xt_sb[:, j * C : (j + 1) * C],
                                 m[:, :],
                                 start=(j == 0), stop=(j == NT - 1))

            def finish_mlp(c):
                sl = slice(c * 512, (c + 1) * 512)
                xe_t = work.tile([C, 512], F32, tag="xe", bufs=2, name=f"xe_{c}")
                nc.gpsimd.tensor_scalar(out=xe_t[:, :], in0=xbc[c][:, :],
                                        scalar1=eps1, scalar2=None, op0=ALU.mult)
                h_t = work.tile([C, 512], F32R, tag="hc", bufs=3, name=f"h_{c}")
                nc.vector.tensor_tensor(out=h_t[:, :], in0=xe_t[:, :],
                                        in1=aggr_tiles[c][:, :], op=ALU.add)
                h1_ps = ps_v.tile([O, 512], F32, tag="v_ps", name=f"h1ps_{c}")
                nc.tensor.matmul(h1_ps[:, :], w1r[:, :], h_t[:, :],
                                 start=True, stop=True)
                nc.scalar.activation(h1c[c][:, :], h1_ps[:, :], AF.Identity,
                                     bias=vecs_sb[:, 0:1],
                                     accum_out=bnsum[:, c : c + 1])
                sq_t = work.tile([O, 512], F32, tag="sqscr", bufs=2, name=f"sq_{c}")
                nc.scalar.activation(sq_t[:, :], h1c[c][:, :], AF.Square,
                                     accum_out=bnsq[:, c : c + 1])

            # software pipeline: iteration it runs phase-1 stripes of chunk
            # it and phase-2 of chunk it-1, interleaved 1:1 on the PE stream.
            for it in range(NCH + 1):
                c1 = it if it < NCH else None
                c2 = it - 1 if it >= 1 else None
                if c2 is not None:
                    aggr_tiles[c2] = ps_a.tile([O, 512], F32, tag="aggr",
                                               name=f"aggr_{c2}")
                cands = {}
                masks = {}
                for k in range(NT):
                    if c1 is not None:
                        s = 4 * c1 + k // 8
                        if k % 8 == 0:
                            cands[s] = work.tile([128, 64], F32, tag="cand",
                                                 bufs=5, name=f"cand_{s}")
                        p1_mm(s, k % 8, cands[s])
                    if c2 is not None:
                        masks[k] = p2_v(c2, k)
                        if k >= 2:
                            p2_aggr(c2, k - 2, masks.pop(k - 2))
                if c1 is not None:
                    # tails only after every stripe's XLc reads are emitted:
                    # the tau DMAs then order cleanly behind them.
                    for kk in range(4):
                        s = 4 * c1 + kk
                        p1_tail(s, cands[s])
                    nc.sync.dma_start(
                        XLc[c1][C + 2 : C + 4, :],
                        tau_scr[c1 * 512 : (c1 + 1) * 512, 0:2].rearrange(
                            "p two -> two p"))
                if c2 is not None:
                    p2_aggr(c2, NT - 2, masks.pop(NT - 2))
                    p2_aggr(c2, NT - 1, masks.pop(NT - 1))
                    finish_mlp(c2)

            # ---------------- BN combine + AllReduce + GELU + W2 ---------
            stats = work.tile([O, 2], F32)
            nc.vector.reduce_sum(stats[:, 0:1], bnsum[:, :], axis=mybir.AxisListType.X)
            nc.vector.reduce_sum(stats[:, 1:2], bnsq[:, :], axis=mybir.AxisListType.X)

            cc_in = dpool.tile([O, 2], F32)
            cc_out = dpool.tile([O, 2], F32, addr_space="Shared")
            nc.sync.dma_start(cc_in[:, :], stats[:, :])
            nc.gpsimd.collective_compute(
                "AllReduce", ALU.add,
                ins=[cc_in[:, :]],
                outs=[cc_out[:, :]],
                replica_groups=[list(range(N_CORES))],
            )
            gstats = work.tile([O, 2], F32)
            nc.sync.dma_start(gstats[:, :], cc_out[:, :])

            mean = work.tile([O, 1], F32)
            var = work.tile([O, 1], F32)
            scale = work.tile([O, 1], F32)
            shift = work.tile([O, 1], F32)
            tmp = work.tile([O, 1], F32)
            nc.vector.tensor_scalar(out=mean[:, :], in0=gstats[:, 0:1],
                                    scalar1=1.0 / BN_COUNT, scalar2=None, op0=ALU.mult)
            nc.vector.tensor_scalar(out=var[:, :], in0=gstats[:, 1:2],
                                    scalar1=1.0 / BN_COUNT, scalar2=None, op0=ALU.mult)
            nc.vector.tensor_tensor(out=tmp[:, :], in0=mean[:, :], in1=mean[:, :],
                                    op=ALU.mult)
            nc.vector.tensor_tensor(out=var[:, :], in0=var[:, :], in1=tmp[:, :],
                                    op=ALU.subtract)
            nc.vector.tensor_scalar(out=var[:, :], in0=var[:, :], scalar1=BN_EPS,
                                    scalar2=None, op0=ALU.add)
            nc.scalar.activation(tmp[:, :], var[:, :], AF.Sqrt)
            nc.vector.reciprocal(out=tmp[:, :], in_=tmp[:, :])
            nc.vector.tensor_tensor(out=scale[:, :], in0=vecs_sb[:, 1:2],
                                    in1=tmp[:, :], op=ALU.mult)
            nc.vector.tensor_tensor(out=tmp[:, :], in0=mean[:, :], in1=scale[:, :],
                                    op=ALU.mult)
            nc.vector.tensor_tensor(out=shift[:, :], in0=vecs_sb[:, 2:3],
                                    in1=tmp[:, :], op=ALU.subtract)

            hg = work.tile([O, 512], F32R, tag="hg", bufs=3)
            y_sb = work.tile([O, 512], F32, tag="ysb", bufs=3)
            for c in range(NCH):
                sl = slice(c * 512, (c + 1) * 512)
                hg_t = work.tile([O, 512], F32R, tag="hg", bufs=3, name=f"hg_{c}")
                nc.scalar.activation(hg_t[:, :], h1c[c][:, :], AF.Gelu,
                                     scale=scale[:, :], bias=shift[:, :])
                o_ps = ps_v.tile([O, 512], F32, tag="v_ps", name=f"ops_{c}")
                nc.tensor.matmul(o_ps[:, :], w2r[:, :], hg_t[:, :],
                                 start=True, stop=True)
                y_t = work.tile([O, 512], F32, tag="ysb", bufs=3, name=f"y_{c}")
                nc.vector.tensor_scalar(out=y_t[:, :], in0=o_ps[:, :],
                                        scalar1=vecs_sb[:, 3:4], scalar2=None,
                                        op0=ALU.add)
                nc.sync.dma_start(y_d[:, sl], y_t[:, :])

            for cm in reversed(_cms):
                cm.__exit__(None, None, None)

    if not nc.is_finalized():
        nc.finalize()
    return nc


def _get_runner():
    """Build the Bass module once and cache a jitted 8-core executable."""
    if "runner" in _cache:
        return _cache["runner"]

    import jax
    import concourse.mybir as mb
    from jax.sharding import Mesh, PartitionSpec
    from jax.experimental.shard_map import shard_map
    from concourse import bass2jax

    nc = _build()
    bass2jax.install_neuronx_cc_hook()

    partition_name = nc.partition_id_tensor.name if nc.partition_id_tensor else None
    in_names = []
    out_names = []
    out_avals = []
    for alloc in nc.m.functions[0].allocations:
        if not isinstance(alloc, mb.MemoryLocationSet):
            continue
        name = alloc.memorylocations[0].name
        if alloc.kind == "ExternalInput":
            if name != partition_name:
                in_names.append(name)
        elif alloc.kind == "ExternalOutput":
            out_names.append(name)
            out_avals.append(
                jax.core.ShapedArray(tuple(alloc.tensor_shape), mb.dt.np(alloc.dtype))
            )
    n_params = len(in_names)
    all_in_names = list(in_names)
    if partition_name is not None:
        all_in_names = all_in_names + [partition_name]

    def _body(*args):
        operands = list(args)
        if partition_name is not None:
            operands.append(bass2jax.partition_id_tensor())
        outs = bass2jax._bass_exec_p.bind(
            *operands,
            out_avals=tuple(out_avals),
            in_names=tuple(all_in_names),
            out_names=tuple(out_names),
            lowering_input_output_aliases=(),
            sim_require_finite=True,
            sim_require_nnan=True,
            nc=nc,
        )
        return tuple(outs)

    devices = jax.devices()[:N_CORES]
    assert len(devices) == N_CORES, f"need {N_CORES} devices, have {len(jax.devices())}"
    mesh = Mesh(np.asarray(devices), ("core",))
    n_outs = len(out_names)
    sharded = jax.jit(
        shard_map(
            _body,
            mesh=mesh,
            in_specs=(PartitionSpec("core"),) * n_params,
            out_specs=(PartitionSpec("core"),) * n_outs,
            check_rep=False,
        ),
        keep_unused=True,
    )
    _cache["runner"] = (sharded, in_names, out_names, out_avals)
    return _cache["runner"]


def kernel(**inputs) -> np.ndarray:
    x = np.asarray(inputs["x"], dtype=np.float32)
    assert x.shape == (B, C, N, 1), x.shape
    k = int(np.asarray(inputs.get("k", K_NN)))
    assert k == K_NN, f"kernel compiled for k={K_NN}, got {k}"
    w1 = np.asarray(inputs["w1"], dtype=np.float32)
    b1 = np.asarray(inputs["b1"], dtype=np.float32)
    gamma = np.asarray(inputs["gamma"], dtype=np.float32)
    beta = np.asarray(inputs["beta"], dtype=np.float32)
    w2 = np.asarray(inputs["w2"], dtype=np.float32)
    b2 = np.asarray(inputs["b2"], dtype=np.float32)
    eps_gin = float(np.asarray(inputs["eps_gin"]))

    sharded, in_names, out_names, out_avals = _get_runner()

    xb = np.ascontiguousarray(x[:, :, :, 0])                     # [B, C, N]
    vecs = np.stack(
        [b1, gamma, beta, b2, np.full(O, 1.0 + eps_gin, np.float32)], axis=1
    ).astype(np.float32)                                         # [64, 5]
    ones2 = np.ones((2, N), np.float32)
    ones_col = np.ones((C, 1), np.float32)
    identr = np.eye(C, dtype=np.float32)

    per_core = {
        "xb": xb,
        "w1": np.broadcast_to(w1, (N_CORES,) + w1.shape),
        "w2": np.broadcast_to(w2, (N_CORES,) + w2.shape),
        "vecs": np.broadcast_to(vecs, (N_CORES,) + vecs.shape),
        "ones2": np.broadcast_to(ones2, (N_CORES,) + ones2.shape),
        "ones_col": np.broadcast_to(ones_col, (N_CORES,) + ones_col.shape),
        "identr": np.broadcast_to(identr, (N_CORES,) + identr.shape),
    }
    concat_in = [
        np.ascontiguousarray(per_core[name]).reshape(
            (N_CORES * per_core[name].shape[1],) + per_core[name].shape[2:]
        )
        for name in in_names
    ]
    out_arrs = sharded(*concat_in)
    yi = out_names.index("y")
    y = np.asarray(out_arrs[yi]).reshape(N_CORES, O, N)
    return y[..., None].astype(np.float32)


# revision 22
# speedup vs baseline: 1.8384x; 1.0636x over previous
"""DyGraphGIN2d Trainium kernel: kNN graph (k=16) + GIN aggregation + MLP/BN/GELU.

Sharding: data-parallel over batch B=8 across 8 NeuronCores (one batch
element per core). BatchNorm uses training-mode batch statistics over ALL
B*N rows, so per-core partial sums go through one in-kernel AllReduce.

Algorithm per core (N=4096 points, C=64 dims), single-matmul phases:
  The ranking metric s[n,m] = <x_hi_n, x_hi_m> + q_m with q = -|x|^2/2
  carried EXACTLY as two f32r rank-1 rows (q_hi + q_lo, an exact hi/lo
  split), all folded into ONE K=66 f32r matmul per 128x512 block (extra
  contraction rows are free: matmul cost is output-columns only).
  Phase 1 (tau): per 128-row stripe, 8 such matmuls + DVE top-8 per
  512-chunk + max/match_replace/max give the 16th-largest s per row;
  tau rides back into the XL operand as two more exact f32r rank-1 rows
  (-tau_hi, -tau_lo).
  Phase 2 (mask+aggregate): v' = s^T - tau is recomputed transposed by the
  mirrored K=68 matmul -- the first 66 product terms are bitwise identical
  to phase 1 (commuted multiplies, same PSUM order), so selection is
  bit-consistent; the 1e-5 guard inside tau makes v' > 0 strict for the
  16 selected neighbors. Masks {0,1} are made OFF the DVE: ACT computes
  sigmoid(4e6 * v') (saturates to exactly 1.0/0.0 in f32r) and Pool(GPSIMD)
  computes is_gt(v', 0), alternating per block. aggr[c,n] accumulates in
  PSUM via single-pass f32r matmuls with x_hi^T (PE-transposed).
  The phase-1 stripes (DVE-bound) and phase-2 blocks (PE-bound) are
  software-pipelined chunk-by-chunk so PE/DVE/ACT/Pool all stay busy.
  Tail: h = (1+eps)x + aggr (Pool); h1 = W1^T h + b1 (f32r PE + ACT bias
  with accum_out BN sums); BN stats AllReduce; fused BN+erf-GELU on ACT;
  out = W2^T hg + b2.

The jitted 8-core shard_map executable is cached across kernel() calls.
"""

import numpy as np

import concourse.bacc as bacc
import concourse.mybir as mybir
from concourse.tile import TileContext

F32 = mybir.dt.float32
F32R = mybir.dt.float32r
AF = mybir.ActivationFunctionType
ALU = mybir.AluOpType

B, C, N, O = 8, 64, 4096, 64
K_NN = 16
N_CORES = 8
NT = N // 128          # 32 row stripes
NCH = 8                # 512-wide column chunks
BN_EPS = 1e-5
BN_COUNT = float(B * N)
TAU_GUARD = 1e-5
SIG_SCALE = 4e6

_cache = {}


def _build():
    nc = bacc.Bacc("TRN2", target_bir_lowering=False)

    xb_d = nc.dram_tensor("xb", [C, N], F32, kind="ExternalInput")
    w1_d = nc.dram_tensor("w1", [C, O], F32, kind="ExternalInput")
    w2_d = nc.dram_tensor("w2", [O, O], F32, kind="ExternalInput")
    vecs_d = nc.dram_tensor("vecs", [O, 5], F32, kind="ExternalInput")  # b1,gamma,beta,b2,eps1
    ones2_d = nc.dram_tensor("ones2", [2, N], F32R, kind="ExternalInput")
    identr_d = nc.dram_tensor("identr", [C, C], F32R, kind="ExternalInput")
    y_d = nc.dram_tensor("y", [O, N], F32, kind="ExternalOutput")
    tau_scr = nc.dram_tensor("tau_scr", [N, 2], F32R)  # internal scratch

    with TileContext(nc) as tc:
        with tc.tile_pool(name="big", bufs=1) as big, \
             tc.tile_pool(name="work", bufs=1) as work, \
             tc.tile_pool(name="dram", bufs=1, space="DRAM") as dpool:

            # ---------------- prologue: operands ----------------
            vecs_sb = work.tile([O, 5], F32)
            w1_sb = work.tile([C, O], F32)
            w2_sb = work.tile([O, O], F32)
            identr = work.tile([C, C], F32R)
            nc.sync.dma_start(vecs_sb[:, :], vecs_d[:, :])
            nc.sync.dma_start(w1_sb[:, :], w1_d[:, :])
            nc.sync.dma_start(w2_sb[:, :], w2_d[:, :])
            nc.sync.dma_start(identr[:, :], identr_d[:, :])
            w1r = work.tile([C, O], F32R)
            w2r = work.tile([O, O], F32R)
            nc.scalar.activation(w1r[:, :], w1_sb[:, :], AF.Copy)
            nc.scalar.activation(w2r[:, :], w2_sb[:, :], AF.Copy)

            xbc = [big.tile([C, 512], F32, name=f"xbc{i}") for i in range(NCH)]
            # XLc: p1 lhsT rows [x_hi; 1; 1], p2 rhs rows [x_hi; 1; 1;
            # -tau_hi; -tau_lo].  The tau rows are DMA-written only after all
            # p1 stripes of the chunk have been emitted, so no false
            # whole-tile WAR stalls arise.
            XLc = [big.tile([128, 512], F32R, name=f"XLc{i}") for i in range(NCH)]
            XRc = [big.tile([128, 512], F32R, name=f"XRc{i}") for i in range(NCH)]
            xt_sb = big.tile([128, NT * C], F32R)

            import concourse.bass_isa as bass_isa
            xsq = work.tile([C, 512], F32, tag="xsq", bufs=2)
            sqall = work.tile([C, 512], F32, tag="sqall", bufs=2)
            qt = work.tile([1, 512], F32, tag="qt", bufs=2)
            for c in range(NCH):
                sl = slice(c * 512, (c + 1) * 512)
                nc.sync.dma_start(xbc[c][:, :], xb_d[:, sl])
                nc.scalar.activation(XLc[c][:C, :], xbc[c][:, :], AF.Copy)
                nc.sync.dma_start(XLc[c][C : C + 2, :], ones2_d[:, sl])
                nc.gpsimd.tensor_copy(XRc[c][:C, :], XLc[c][:C, :])
                nc.sync.dma_start(XRc[c][C + 2 : C + 4, :], ones2_d[:, sl])
                # q = -|x_m|^2/2 exactly as f32r hi+lo rank-1 rows; the
                # partition reduce runs on the idle GPSIMD engine (cold-PE
                # fp32 matmuls here cost ~2.4us each at low p-state).
                xsq_t = work.tile([C, 512], F32, tag="xsq", bufs=2, name=f"xsq_{c}")
                sq_t = work.tile([C, 512], F32, tag="sqall", bufs=2, name=f"sqa_{c}")
                nc.gpsimd.tensor_tensor(out=xsq_t[:, :], in0=xbc[c][:, :],
                                        in1=xbc[c][:, :], op=ALU.mult)
                nc.gpsimd.partition_all_reduce(sq_t[:, :], xsq_t[:, :],
                                               channels=C,
                                               reduce_op=bass_isa.ReduceOp.add)
                nc.scalar.activation(XRc[c][C : C + 1, :], sq_t[0:1, :], AF.Copy,
                                     scale=-0.5)
                qh_t = work.tile([1, 512], F32R, tag="qh", bufs=2, name=f"qh_{c}")
                nc.scalar.activation(qh_t[:, :], sq_t[0:1, :], AF.Copy, scale=-0.5)
                qt_t = work.tile([1, 512], F32, tag="qt", bufs=2, name=f"qt_{c}")
                nc.vector.tensor_scalar(out=qt_t[:, :], in0=sq_t[0:1, :],
                                        scalar1=-0.5, scalar2=None, op0=ALU.mult)
                nc.vector.tensor_tensor(out=qt_t[:, :], in0=qt_t[:, :],
                                        in1=qh_t.bitcast(F32)[:, :],
                                        op=ALU.subtract)
                # ACT can only write at partition base 0/64; q_lo (row 65)
                # goes through a partition-0 staging tile + DMA.
                ql_t = work.tile([1, 512], F32R, tag="ql", bufs=2,
                                 name=f"ql_{c}")
                nc.scalar.activation(ql_t[:, :], qt_t[:, :], AF.Copy)
                nc.sync.dma_start(XRc[c][C + 1 : C + 2, :], ql_t[:, :])

            # x_hi^T chunks for the aggregation matmuls (PE transpose)
            with tc.tile_pool(name="ps_tp", bufs=2, space="PSUM") as ps_tp:
                for j in range(NT):
                    tp = ps_tp.tile([128, C], F32R, tag="tp_ps")
                    nc.tensor.transpose(tp[:, :],
                                        XLc[j // 4][:C, (j % 4) * 128 : (j % 4 + 1) * 128],
                                        identr[:, :])
                    nc.scalar.activation(xt_sb[:, j * C : (j + 1) * C],
                                         tp[:, :], AF.Copy)

            # ---------------- main pipelined loop ----------------
            ps_s_cm = tc.tile_pool(name="ps_s", bufs=3, space="PSUM")
            ps_v_cm = tc.tile_pool(name="ps_v", bufs=3, space="PSUM")
            ps_a_cm = tc.tile_pool(name="ps_a", bufs=2, space="PSUM")
            ps_s = ps_s_cm.__enter__()
            ps_v = ps_v_cm.__enter__()
            ps_a = ps_a_cm.__enter__()
            _cms = [ps_s_cm, ps_v_cm, ps_a_cm]

            cand = work.tile([128, 64], F32, tag="cand", bufs=5)
            t8a = work.tile([128, 8], F32, tag="t8a", bufs=4)
            t8b = work.tile([128, 8], F32, tag="t8b", bufs=4)
            ntf = work.tile([128, 1], F32, tag="ntf", bufs=4)
            ntau2 = work.tile([128, 2], F32R, tag="ntau2", bufs=4)
            mask = work.tile([128, 512], F32R, tag="mask", bufs=6)
            hc = work.tile([C, 512], F32R, tag="hc", bufs=3)
            h1c = [big.tile([O, 512], F32, name=f"h1c{i}") for i in range(NCH)]
            bnsum = work.tile([O, NCH], F32)
            bnsq = work.tile([O, NCH], F32)
            sqscr = work.tile([O, 512], F32, tag="sqscr", bufs=2)
            eps1 = vecs_sb[:, 4:5]

            aggr_tiles = {}

            def p1_mm(s, c8, cand_t):
                jt, jo = s // 4, (s % 4) * 128
                s_ps = ps_s.tile([128, 512], F32, tag="s_ps", name=f"s_{s}_{c8}")
                nc.tensor.matmul(s_ps[:, :], XLc[jt][: C + 2, jo : jo + 128],
                                 XRc[c8][: C + 2, :], start=True, stop=True)
                nc.vector.max(out=cand_t[:, c8 * 8 : (c8 + 1) * 8], in_=s_ps[:, :])

            def p1_tail(s, cand_t):
                jt, jo = s // 4, (s % 4) * 128
                t8a_t = work.tile([128, 8], F32, tag="t8a", bufs=4, name=f"t8a_{s}")
                t8b_t = work.tile([128, 8], F32, tag="t8b", bufs=4, name=f"t8b_{s}")
                ntf_t = work.tile([128, 1], F32, tag="ntf", bufs=4, name=f"ntf_{s}")
                nt2_t = work.tile([128, 2], F32R, tag="ntau2", bufs=4, name=f"nt2_{s}")
                nc.vector.max(out=t8a_t[:, :], in_=cand_t[:, :])
                nc.vector.match_replace(out=cand_t[:, :], in_to_replace=t8a_t[:, :],
                                        in_values=cand_t[:, :], imm_value=-1e30)
                nc.vector.max(out=t8b_t[:, :], in_=cand_t[:, :])
                # -tau = -(t16 - guard) = guard - t16, split exactly hi+lo
                nc.gpsimd.tensor_scalar(out=ntf_t[:, :], in0=t8b_t[:, 7:8],
                                        scalar1=-1.0, scalar2=TAU_GUARD,
                                        op0=ALU.mult, op1=ALU.add)
                nc.vector.tensor_copy(nt2_t[:, 0:1], ntf_t[:, :])
                nc.gpsimd.tensor_tensor(out=nt2_t.bitcast(F32)[:, 1:2],
                                        in0=ntf_t[:, :],
                                        in1=nt2_t.bitcast(F32)[:, 0:1],
                                        op=ALU.subtract)
                # SBUF->SBUF DMA cannot transpose partition->free; bounce
                # the per-stripe [128,2] tau pair through flat DRAM.  The
                # chunk-wide readback into XLc happens in tau_readback().
                nc.sync.dma_start(tau_scr[s * 128 : (s + 1) * 128, :], nt2_t[:, 0:2])

            def p2_v(c, j):
                jt, jo = j // 4, (j % 4) * 128
                v_ps = ps_v.tile([128, 512], F32, tag="v_ps", name=f"v_{c}_{j}")
                nc.tensor.matmul(v_ps[:, :], XRc[jt][: C + 4, jo : jo + 128],
                                 XLc[c][: C + 4, :], start=True, stop=True)
                m = work.tile([128, 512], F32R, tag="mask", bufs=6, name=f"m_{c}_{j}")
                if c == NCH - 1:
                    nc.vector.tensor_scalar(out=m[:, :], in0=v_ps[:, :],
                                            scalar1=0.0, scalar2=None,
                                            op0=ALU.is_gt)
                else:
                    nc.scalar.activation(m[:, :], v_ps[:, :], AF.Sigmoid,
                                         scale=SIG_SCALE)
                return m

            def p2_aggr(c, j, m):
                nc.tensor.matmul(aggr_tiles[c][:, :],
                                 xt_sb[:, j * C : (j + 1) * C],
                                 m[:, :],
                                 start=(j == 0), stop=(j == NT - 1))

            def finish_mlp(c):
                sl = slice(c * 512, (c + 1) * 512)
                xe_t = work.tile([C, 512], F32, tag="xe", bufs=2, name=f"xe_{c}")
                nc.gpsimd.tensor_scalar(out=xe_t[:, :], in0=xbc[c][:, :],
                                        scalar1=eps1, scalar2=None, op0=ALU.mult)
                h_t = work.tile([C, 512], F32R, tag="hc", bufs=3, name=f"h_{c}")
                nc.vector.tensor_tensor(out=h_t[:, :], in0=xe_t[:, :],
                                        in1=aggr_tiles[c][:, :], op=ALU.add)
                h1_ps = ps_v.tile([O, 512], F32, tag="v_ps", name=f"h1ps_{c}")
                nc.tensor.matmul(h1_ps[:, :], w1r[:, :], h_t[:, :],
                                 start=True, stop=True)
                nc.scalar.activation(h1c[c][:, :], h1_ps[:, :], AF.Identity,
                                     bias=vecs_sb[:, 0:1],
                                     accum_out=bnsum[:, c : c + 1])
                sq_t = work.tile([O, 512], F32, tag="sqscr", bufs=2, name=f"sq_{c}")
                nc.scalar.activation(sq_t[:, :], h1c[c][:, :], AF.Square,
                                     accum_out=bnsq[:, c : c + 1])

            # software pipeline: iteration it runs phase-1 stripes of chunk
            # it and phase-2 of chunk it-1, interleaved 1:1 on the PE stream.
            for it in range(NCH + 1):
                c1 = it if it < NCH else None
                c2 = it - 1 if it >= 1 else None
                if c2 is not None:
                    aggr_tiles[c2] = ps_a.tile([O, 512], F32, tag="aggr",
                                               name=f"aggr_{c2}")
                cands = {}
                masks = {}
                for k in range(NT):
                    if c1 is not None:
                        s = 4 * c1 + k // 8
                        if k % 8 == 0:
                            cands[s] = work.tile([128, 64], F32, tag="cand",
                                                 bufs=5, name=f"cand_{s}")
                        p1_mm(s, k % 8, cands[s])
                    if c2 is not None:
                        masks[k] = p2_v(c2, k)
                        if k >= 2:
                            p2_aggr(c2, k - 2, masks.pop(k - 2))
                if c1 is not None:
                    # tails only after every stripe's XLc reads are emitted:
                    # the tau DMAs then order cleanly behind them.
                    for kk in range(4):
                        s = 4 * c1 + kk
                        p1_tail(s, cands[s])
                    nc.sync.dma_start(
                        XLc[c1][C + 2 : C + 4, :],
                        tau_scr[c1 * 512 : (c1 + 1) * 512, 0:2].rearrange(
                            "p two -> two p"))
                if c2 is not None:
                    p2_aggr(c2, NT - 2, masks.pop(NT - 2))
                    p2_aggr(c2, NT - 1, masks.pop(NT - 1))
                    finish_mlp(c2)

            # ---------------- BN combine + AllReduce + GELU + W2 ---------
            stats = work.tile([O, 2], F32)
            nc.vector.reduce_sum(stats[:, 0:1], bnsum[:, :], axis=mybir.AxisListType.X)
            nc.vector.reduce_sum(stats[:, 1:2], bnsq[:, :], axis=mybir.AxisListType.X)

            # AllGather + local 8-way add: the collective cost model charges
            # AllReduce 1.875x the fixed ~15us latency, AllGather 1x.
            cc_in = dpool.tile([O, 2], F32)
            cc_out = dpool.tile([N_CORES * O, 2], F32, addr_space="Shared")
            nc.sync.dma_start(cc_in[:, :], stats[:, :])
            nc.gpsimd.collective_compute(
                "AllGather", ALU.bypass,
                ins=[cc_in[:, :]],
                outs=[cc_out[:, :]],
                replica_groups=[list(range(N_CORES))],
            )
            gall = work.tile([O, 2 * N_CORES], F32)
            nc.sync.dma_start(
                gall[:, :].rearrange("p (two k) -> p two k", two=2, k=N_CORES),
                cc_out[:, :].rearrange("(k p) two -> p two k", k=N_CORES, p=O))
            gstats = work.tile([O, 2], F32)
            nc.vector.reduce_sum(
                gstats[:, 0:2],
                gall[:, :].rearrange("p (two k) -> p two k", two=2, k=N_CORES),
                axis=mybir.AxisListType.X)

            mean = work.tile([O, 1], F32)
            var = work.tile([O, 1], F32)
            scale = work.tile([O, 1], F32)
            shift = work.tile([O, 1], F32)
            tmp = work.tile([O, 1], F32)
            nc.vector.tensor_scalar(out=mean[:, :], in0=gstats[:, 0:1],
                                    scalar1=1.0 / BN_COUNT, scalar2=None, op0=ALU.mult)
            nc.vector.tensor_scalar(out=var[:, :], in0=gstats[:, 1:2],
                                    scalar1=1.0 / BN_COUNT, scalar2=None, op0=ALU.mult)
            nc.vector.tensor_tensor(out=tmp[:, :], in0=mean[:, :], in1=mean[:, :],
                                    op=ALU.mult)
            nc.vector.tensor_tensor(out=var[:, :], in0=var[:, :], in1=tmp[:, :],
                                    op=ALU.subtract)
            nc.vector.tensor_scalar(out=var[:, :], in0=var[:, :], scalar1=BN_EPS,
                                    scalar2=None, op0=ALU.add)
            nc.scalar.activation(tmp[:, :], var[:, :], AF.Sqrt)
            nc.vector.reciprocal(out=tmp[:, :], in_=tmp[:, :])
            nc.vector.tensor_tensor(out=scale[:, :], in0=vecs_sb[:, 1:2],
                                    in1=tmp[:, :], op=ALU.mult)
            nc.vector.tensor_tensor(out=tmp[:, :], in0=mean[:, :], in1=scale[:, :],
                                    op=ALU.mult)
            nc.vector.tensor_tensor(out=shift[:, :], in0=vecs_sb[:, 2:3],
                                    in1=tmp[:, :], op=ALU.subtract)

            hg = work.tile([O, 512], F32R, tag="hg", bufs=3)
            y_sb = work.tile([O, 512], F32, tag="ysb", bufs=3)
            for c in range(NCH):
                sl = slice(c * 512, (c + 1) * 512)
                hg_t = work.tile([O, 512], F32R, tag="hg", bufs=3, name=f"hg_{c}")
                nc.scalar.activation(hg_t[:, :], h1c[c][:, :], AF.Gelu,
                                     scale=scale[:, :], bias=shift[:, :])
                o_ps = ps_v.tile([O, 512], F32, tag="v_ps", name=f"ops_{c}")
                nc.tensor.matmul(o_ps[:, :], w2r[:, :], hg_t[:, :],
                                 start=True, stop=True)
                y_t = work.tile([O, 512], F32, tag="ysb", bufs=3, name=f"y_{c}")
                nc.vector.tensor_scalar(out=y_t[:, :], in0=o_ps[:, :],
                                        scalar1=vecs_sb[:, 3:4], scalar2=None,
                                        op0=ALU.add)
                nc.sync.dma_start(y_d[:, sl], y_t[:, :])

            for cm in reversed(_cms):
                cm.__exit__(None, None, None)

    if not nc.is_finalized():
        nc.finalize()
    return nc


def _get_runner():
    """Build the Bass module once and cache a jitted 8-core executable."""
    if "runner" in _cache:
        return _cache["runner"]

    import jax
    import concourse.mybir as mb
    from jax.sharding import Mesh, PartitionSpec
    from jax.experimental.shard_map import shard_map
    from concourse import bass2jax

    nc = _build()
    bass2jax.install_neuronx_cc_hook()

    partition_name = nc.partition_id_tensor.name if nc.partition_id_tensor else None
    in_names = []
    out_names = []
    out_avals = []
    for alloc in nc.m.functions[0].allocations:
        if not isinstance(alloc, mb.MemoryLocationSet):
            continue
        name = alloc.memorylocations[0].name
        if alloc.kind == "ExternalInput":
            if name != partition_name:
                in_names.append(name)
        elif alloc.kind == "ExternalOutput":
            out_names.append(name)
            out_avals.append(
                jax.core.ShapedArray(tuple(alloc.tensor_shape), mb.dt.np(alloc.dtype))
            )
    n_params = len(in_names)
    all_in_names = list(in_names)
    if partition_name is not None:
        all_in_names = all_in_names + [partition_name]

    def _body(*args):
        operands = list(args)
        if partition_name is not None:
            operands.append(bass2jax.partition_id_tensor())
        outs = bass2jax._bass_exec_p.bind(
            *operands,
            out_avals=tuple(out_avals),
            in_names=tuple(all_in_names),
            out_names=tuple(out_names),
            lowering_input_output_aliases=(),
            sim_require_finite=True,
            sim_require_nnan=True,
            nc=nc,
        )
        return tuple(outs)

    devices = jax.devices()[:N_CORES]
    assert len(devices) == N_CORES, f"need {N_CORES} devices, have {len(jax.devices())}"
    mesh = Mesh(np.asarray(devices), ("core",))
    n_outs = len(out_names)
    sharded = jax.jit(
        shard_map(
            _body,
            mesh=mesh,
            in_specs=(PartitionSpec("core"),) * n_params,
            out_specs=(PartitionSpec("core"),) * n_outs,
            check_rep=False,
        ),
        keep_unused=True,
    )
    _cache["runner"] = (sharded, in_names, out_names, out_avals)
    return _cache["runner"]


def kernel(**inputs) -> np.ndarray:
    x = np.asarray(inputs["x"], dtype=np.float32)
    assert x.shape == (B, C, N, 1), x.shape
    k = int(np.asarray(inputs.get("k", K_NN)))
    assert k == K_NN, f"kernel compiled for k={K_NN}, got {k}"
    w1 = np.asarray(inputs["w1"], dtype=np.float32)
    b1 = np.asarray(inputs["b1"], dtype=np.float32)
    gamma = np.asarray(inputs["gamma"], dtype=np.float32)
    beta = np.asarray(inputs["beta"], dtype=np.float32)
    w2 = np.asarray(inputs["w2"], dtype=np.float32)
    b2 = np.asarray(inputs["b2"], dtype=np.float32)
    eps_gin = float(np.asarray(inputs["eps_gin"]))

    sharded, in_names, out_names, out_avals = _get_runner()

    xb = np.ascontiguousarray(x[:, :, :, 0])                     # [B, C, N]
    vecs = np.stack(
        [b1, gamma, beta, b2, np.full(O, 1.0 + eps_gin, np.float32)], axis=1
    ).astype(np.float32)                                         # [64, 5]
    ones2 = np.ones((2, N), np.float32)
    ones_col = np.ones((C, 1), np.float32)
    identr = np.eye(C, dtype=np.float32)

    per_core = {
        "xb": xb,
        "w1": np.broadcast_to(w1, (N_CORES,) + w1.shape),
        "w2": np.broadcast_to(w2, (N_CORES,) + w2.shape),
        "vecs": np.broadcast_to(vecs, (N_CORES,) + vecs.shape),
        "ones2": np.broadcast_to(ones2, (N_CORES,) + ones2.shape),
        "ones_col": np.broadcast_to(ones_col, (N_CORES,) + ones_col.shape),
        "identr": np.broadcast_to(identr, (N_CORES,) + identr.shape),
    }
    concat_in = [
        np.ascontiguousarray(per_core[name]).reshape(
            (N_CORES * per_core[name].shape[1],) + per_core[name].shape[2:]
        )
        for name in in_names
    ]
    out_arrs = sharded(*concat_in)
    yi = out_names.index("y")
    y = np.asarray(out_arrs[yi]).reshape(N_CORES, O, N)
    return y[..., None].astype(np.float32)


# revision 23
# speedup vs baseline: 1.9611x; 1.0667x over previous
"""DyGraphGIN2d Trainium kernel: kNN graph (k=16) + GIN aggregation + MLP/BN/GELU.

Sharding: data-parallel over batch B=8 across 8 NeuronCores (one batch
element per core). BatchNorm uses training-mode batch statistics over ALL
B*N rows, so per-core partial sums go through one in-kernel AllReduce.

Algorithm per core (N=4096 points, C=64 dims), single-matmul phases:
  The ranking metric s[n,m] = <x_hi_n, x_hi_m> + q_m with q = -|x|^2/2
  carried EXACTLY as two f32r rank-1 rows (q_hi + q_lo, an exact hi/lo
  split), all folded into ONE K=66 f32r matmul per 128x512 block (extra
  contraction rows are free: matmul cost is output-columns only).
  Phase 1 (tau): per 128-row stripe, 8 such matmuls + DVE top-8 per
  512-chunk + max/match_replace/max give the 16th-largest s per row;
  tau rides back into the XL operand as two more exact f32r rank-1 rows
  (-tau_hi, -tau_lo).
  Phase 2 (mask+aggregate): v' = s^T - tau is recomputed transposed by the
  mirrored K=68 matmul -- the first 66 product terms are bitwise identical
  to phase 1 (commuted multiplies, same PSUM order), so selection is
  bit-consistent; the 1e-5 guard inside tau makes v' > 0 strict for the
  16 selected neighbors. Masks {0,1} are made OFF the DVE: ACT computes
  sigmoid(4e6 * v') (saturates to exactly 1.0/0.0 in f32r) and Pool(GPSIMD)
  computes is_gt(v', 0), alternating per block. aggr[c,n] accumulates in
  PSUM via single-pass f32r matmuls with x_hi^T (PE-transposed).
  The phase-1 stripes (DVE-bound) and phase-2 blocks (PE-bound) are
  software-pipelined chunk-by-chunk so PE/DVE/ACT/Pool all stay busy.
  Tail: h = (1+eps)x + aggr (Pool); h1 = W1^T h + b1 (f32r PE + ACT bias
  with accum_out BN sums); BN stats AllReduce; fused BN+erf-GELU on ACT;
  out = W2^T hg + b2.

The jitted 8-core shard_map executable is cached across kernel() calls.
"""

import numpy as np

import concourse.bacc as bacc
import concourse.mybir as mybir
from concourse.tile import TileContext

F32 = mybir.dt.float32
F32R = mybir.dt.float32r
AF = mybir.ActivationFunctionType
ALU = mybir.AluOpType

B, C, N, O = 8, 64, 4096, 64
K_NN = 16
N_CORES = 8
NT = N // 128          # 32 row stripes
NCH = 8                # 512-wide column chunks
BN_EPS = 1e-5
BN_COUNT = float(B * N)
TAU_GUARD = 1e-5
SIG_SCALE = 4e6

_cache = {}


def _build():
    nc = bacc.Bacc("TRN2", target_bir_lowering=False)

    xb_d = nc.dram_tensor("xb", [C, N], F32, kind="ExternalInput")
    w1_d = nc.dram_tensor("w1", [C, O], F32, kind="ExternalInput")
    w2_d = nc.dram_tensor("w2", [O, O], F32, kind="ExternalInput")
    vecs_d = nc.dram_tensor("vecs", [O, 5], F32, kind="ExternalInput")  # b1,gamma,beta,b2,eps1
    ones2_d = nc.dram_tensor("ones2", [2, N], F32R, kind="ExternalInput")
    identr_d = nc.dram_tensor("identr", [C, C], F32R, kind="ExternalInput")
    y_d = nc.dram_tensor("y", [O, N], F32, kind="ExternalOutput")
    tau_scr = nc.dram_tensor("tau_scr", [N, 2], F32R)  # internal scratch

    with TileContext(nc) as tc:
        with tc.tile_pool(name="big", bufs=1) as big, \
             tc.tile_pool(name="work", bufs=1) as work, \
             tc.tile_pool(name="dram", bufs=1, space="DRAM") as dpool:

            # ---------------- prologue: operands ----------------
            vecs_sb = work.tile([O, 5], F32)
            w1_sb = work.tile([C, O], F32)
            w2_sb = work.tile([O, O], F32)
            identr = work.tile([C, C], F32R)
            nc.sync.dma_start(vecs_sb[:, :], vecs_d[:, :])
            nc.sync.dma_start(w1_sb[:, :], w1_d[:, :])
            nc.sync.dma_start(w2_sb[:, :], w2_d[:, :])
            nc.sync.dma_start(identr[:, :], identr_d[:, :])
            w1r = work.tile([C, O], F32R)
            w2r = work.tile([O, O], F32R)
            eid = work.tile([C, C], F32R)
            nc.scalar.activation(w1r[:, :], w1_sb[:, :], AF.Copy)
            nc.scalar.activation(w2r[:, :], w2_sb[:, :], AF.Copy)
            # diag(1+eps) in f32r: folds the (1+eps)*x_hi term into the
            # aggregation PSUM via one extra matmul per chunk.
            nc.scalar.activation(eid[:, :], identr[:, :], AF.Copy,
                                 scale=vecs_sb[:, 4:5])

            xbc = [big.tile([C, 512], F32, name=f"xbc{i}") for i in range(NCH)]
            # XLc: p1 lhsT rows [x_hi; 1; 1], p2 rhs rows [x_hi; 1; 1;
            # -tau_hi; -tau_lo].  The tau rows are DMA-written only after all
            # p1 stripes of the chunk have been emitted, so no false
            # whole-tile WAR stalls arise.
            XLc = [big.tile([128, 512], F32R, name=f"XLc{i}") for i in range(NCH)]
            XRc = [big.tile([128, 512], F32R, name=f"XRc{i}") for i in range(NCH)]
            xt_sb = big.tile([128, NT * C], F32R)

            import concourse.bass_isa as bass_isa
            xsq = work.tile([C, 512], F32, tag="xsq", bufs=2)
            sqall = work.tile([C, 512], F32, tag="sqall", bufs=2)
            qt = work.tile([1, 512], F32, tag="qt", bufs=2)
            for c in range(NCH):
                sl = slice(c * 512, (c + 1) * 512)
                nc.sync.dma_start(xbc[c][:, :], xb_d[:, sl])
                nc.scalar.activation(XLc[c][:C, :], xbc[c][:, :], AF.Copy)
                nc.sync.dma_start(XLc[c][C : C + 2, :], ones2_d[:, sl])
                nc.gpsimd.tensor_copy(XRc[c][:C, :], XLc[c][:C, :])
                nc.sync.dma_start(XRc[c][C + 2 : C + 4, :], ones2_d[:, sl])
                # q = -|x_m|^2/2 exactly as f32r hi+lo rank-1 rows; the
                # partition reduce runs on the idle GPSIMD engine (cold-PE
                # fp32 matmuls here cost ~2.4us each at low p-state).
                xsq_t = work.tile([C, 512], F32, tag="xsq", bufs=2, name=f"xsq_{c}")
                sq_t = work.tile([C, 512], F32, tag="sqall", bufs=2, name=f"sqa_{c}")
                nc.gpsimd.tensor_tensor(out=xsq_t[:, :], in0=xbc[c][:, :],
                                        in1=xbc[c][:, :], op=ALU.mult)
                nc.gpsimd.partition_all_reduce(sq_t[:, :], xsq_t[:, :],
                                               channels=C,
                                               reduce_op=bass_isa.ReduceOp.add)
                nc.scalar.activation(XRc[c][C : C + 1, :], sq_t[0:1, :], AF.Copy,
                                     scale=-0.5)
                qh_t = work.tile([1, 512], F32R, tag="qh", bufs=2, name=f"qh_{c}")
                nc.scalar.activation(qh_t[:, :], sq_t[0:1, :], AF.Copy, scale=-0.5)
                qt_t = work.tile([1, 512], F32, tag="qt", bufs=2, name=f"qt_{c}")
                nc.vector.tensor_scalar(out=qt_t[:, :], in0=sq_t[0:1, :],
                                        scalar1=-0.5, scalar2=None, op0=ALU.mult)
                nc.vector.tensor_tensor(out=qt_t[:, :], in0=qt_t[:, :],
                                        in1=qh_t.bitcast(F32)[:, :],
                                        op=ALU.subtract)
                # ACT can only write at partition base 0/64; q_lo (row 65)
                # goes through a partition-0 staging tile + DMA.
                ql_t = work.tile([1, 512], F32R, tag="ql", bufs=2,
                                 name=f"ql_{c}")
                nc.scalar.activation(ql_t[:, :], qt_t[:, :], AF.Copy)
                nc.sync.dma_start(XRc[c][C + 1 : C + 2, :], ql_t[:, :])

            # ---------------- main pipelined loop ----------------
            # ps_v/ps_a are entered only after the transpose block below so
            # its PSUM fits; ps_s is needed from iteration 0.
            ps_s_cm = tc.tile_pool(name="ps_s", bufs=3, space="PSUM")
            ps_s = ps_s_cm.__enter__()
            ps_v = ps_a = None
            _cms = [ps_s_cm]

            cand = work.tile([128, 64], F32, tag="cand", bufs=5)
            t8a = work.tile([128, 8], F32, tag="t8a", bufs=4)
            t8b = work.tile([128, 8], F32, tag="t8b", bufs=4)
            ntf = work.tile([128, 1], F32, tag="ntf", bufs=4)
            ntau2 = work.tile([128, 2], F32R, tag="ntau2", bufs=4)
            mask = work.tile([128, 512], F32R, tag="mask", bufs=6)
            hc = work.tile([C, 512], F32R, tag="hc", bufs=3)
            h1c = [big.tile([O, 512], F32, name=f"h1c{i}") for i in range(NCH)]
            bnsum = work.tile([O, NCH], F32)
            bnsq = work.tile([O, NCH], F32)
            sqscr = work.tile([O, 512], F32, tag="sqscr", bufs=2)
            eps1 = vecs_sb[:, 4:5]

            aggr_tiles = {}

            def p1_mm(s, c8, cand_t):
                jt, jo = s // 4, (s % 4) * 128
                s_ps = ps_s.tile([128, 512], F32, tag="s_ps", name=f"s_{s}_{c8}")
                nc.tensor.matmul(s_ps[:, :], XLc[jt][: C + 2, jo : jo + 128],
                                 XRc[c8][: C + 2, :], start=True, stop=True)
                nc.vector.max(out=cand_t[:, c8 * 8 : (c8 + 1) * 8], in_=s_ps[:, :])

            def p1_tail(s, cand_t):
                jt, jo = s // 4, (s % 4) * 128
                t8a_t = work.tile([128, 8], F32, tag="t8a", bufs=4, name=f"t8a_{s}")
                t8b_t = work.tile([128, 8], F32, tag="t8b", bufs=4, name=f"t8b_{s}")
                ntf_t = work.tile([128, 1], F32, tag="ntf", bufs=4, name=f"ntf_{s}")
                nt2_t = work.tile([128, 2], F32R, tag="ntau2", bufs=4, name=f"nt2_{s}")
                nc.vector.max(out=t8a_t[:, :], in_=cand_t[:, :])
                nc.vector.match_replace(out=cand_t[:, :], in_to_replace=t8a_t[:, :],
                                        in_values=cand_t[:, :], imm_value=-1e30)
                nc.vector.max(out=t8b_t[:, :], in_=cand_t[:, :])
                # -tau = -(t16 - guard) = guard - t16, split exactly hi+lo
                nc.gpsimd.tensor_scalar(out=ntf_t[:, :], in0=t8b_t[:, 7:8],
                                        scalar1=-1.0, scalar2=TAU_GUARD,
                                        op0=ALU.mult, op1=ALU.add)
                nc.vector.tensor_copy(nt2_t[:, 0:1], ntf_t[:, :])
                nc.gpsimd.tensor_tensor(out=nt2_t.bitcast(F32)[:, 1:2],
                                        in0=ntf_t[:, :],
                                        in1=nt2_t.bitcast(F32)[:, 0:1],
                                        op=ALU.subtract)
                # SBUF->SBUF DMA cannot transpose partition->free; bounce
                # the per-stripe [128,2] tau pair through flat DRAM.  The
                # chunk-wide readback into XLc happens in tau_readback().
                nc.sync.dma_start(tau_scr[s * 128 : (s + 1) * 128, :], nt2_t[:, 0:2])

            def p2_v(c, j):
                jt, jo = j // 4, (j % 4) * 128
                v_ps = ps_v.tile([128, 512], F32, tag="v_ps", name=f"v_{c}_{j}")
                nc.tensor.matmul(v_ps[:, :], XRc[jt][: C + 4, jo : jo + 128],
                                 XLc[c][: C + 4, :], start=True, stop=True)
                m = work.tile([128, 512], F32R, tag="mask", bufs=6, name=f"m_{c}_{j}")
                if c == NCH - 1 and j % 2 == 0:
                    nc.vector.tensor_scalar(out=m[:, :], in0=v_ps[:, :],
                                            scalar1=0.0, scalar2=None,
                                            op0=ALU.is_gt)
                else:
                    nc.scalar.activation(m[:, :], v_ps[:, :], AF.Sigmoid,
                                         scale=SIG_SCALE)
                return m

            def p2_aggr(c, j, m):
                nc.tensor.matmul(aggr_tiles[c][:, :],
                                 xt_sb[:, j * C : (j + 1) * C],
                                 m[:, :],
                                 start=(j == 0), stop=False)

            def finish_mlp(c):
                sl = slice(c * 512, (c + 1) * 512)
                nc.tensor.matmul(aggr_tiles[c][:, :], eid[:, :], XLc[c][:C, :],
                                 start=False, stop=True)
                h_t = work.tile([C, 512], F32R, tag="hc", bufs=3, name=f"h_{c}")
                nc.scalar.activation(h_t[:, :], aggr_tiles[c][:, :], AF.Copy)
                h1_ps = ps_v.tile([O, 512], F32, tag="v_ps", name=f"h1ps_{c}")
                nc.tensor.matmul(h1_ps[:, :], w1r[:, :], h_t[:, :],
                                 start=True, stop=True)
                nc.scalar.activation(h1c[c][:, :], h1_ps[:, :], AF.Identity,
                                     bias=vecs_sb[:, 0:1],
                                     accum_out=bnsum[:, c : c + 1])
                sq_t = work.tile([O, 512], F32, tag="sqscr", bufs=2, name=f"sq_{c}")
                nc.scalar.activation(sq_t[:, :], h1c[c][:, :], AF.Square,
                                     accum_out=bnsq[:, c : c + 1])

            # software pipeline: iteration it runs phase-1 stripes of chunk
            # it and phase-2 of chunk it-1, interleaved 1:1 on the PE stream.
            for it in range(NCH + 1):
                if it == 1:
                    # x_hi^T chunks for the aggregation matmuls: emitted here
                    # so the PE transposes + ACT copies overlap iteration 0's
                    # DVE-only top-8 work.
                    with tc.tile_pool(name="ps_tp", bufs=2, space="PSUM") as ps_tp:
                        for j in range(NT):
                            tp = ps_tp.tile([128, C], F32R, tag="tp_ps")
                            nc.tensor.transpose(
                                tp[:, :],
                                XLc[j // 4][:C, (j % 4) * 128 : (j % 4 + 1) * 128],
                                identr[:, :])
                            nc.scalar.activation(xt_sb[:, j * C : (j + 1) * C],
                                                 tp[:, :], AF.Copy)
                    ps_v_cm = tc.tile_pool(name="ps_v", bufs=3, space="PSUM")
                    ps_a_cm = tc.tile_pool(name="ps_a", bufs=2, space="PSUM")
                    ps_v = ps_v_cm.__enter__()
                    ps_a = ps_a_cm.__enter__()
                    _cms.extend([ps_v_cm, ps_a_cm])
                c1 = it if it < NCH else None
                c2 = it - 1 if it >= 1 else None
                if c2 is not None:
                    aggr_tiles[c2] = ps_a.tile([O, 512], F32, tag="aggr",
                                               name=f"aggr_{c2}")
                cands = {}
                masks = {}
                for k in range(NT):
                    if c1 is not None:
                        s = 4 * c1 + k // 8
                        if k % 8 == 0:
                            cands[s] = work.tile([128, 64], F32, tag="cand",
                                                 bufs=5, name=f"cand_{s}")
                        p1_mm(s, k % 8, cands[s])
                    if c2 is not None:
                        masks[k] = p2_v(c2, k)
                        if k >= 2:
                            p2_aggr(c2, k - 2, masks.pop(k - 2))
                if c1 is not None:
                    # tails only after every stripe's XLc reads are emitted:
                    # the tau DMAs then order cleanly behind them.
                    for kk in range(4):
                        s = 4 * c1 + kk
                        p1_tail(s, cands[s])
                    nc.sync.dma_start(
                        XLc[c1][C + 2 : C + 4, :],
                        tau_scr[c1 * 512 : (c1 + 1) * 512, 0:2].rearrange(
                            "p two -> two p"))
                if c2 is not None:
                    p2_aggr(c2, NT - 2, masks.pop(NT - 2))
                    p2_aggr(c2, NT - 1, masks.pop(NT - 1))
                    finish_mlp(c2)

            # ---------------- BN combine + AllReduce + GELU + W2 ---------
            stats = work.tile([O, 2], F32)
            nc.vector.reduce_sum(stats[:, 0:1], bnsum[:, :], axis=mybir.AxisListType.X)
            nc.vector.reduce_sum(stats[:, 1:2], bnsq[:, :], axis=mybir.AxisListType.X)

            # AllGather + local 8-way add: the collective cost model charges
            # AllReduce 1.875x the fixed ~15us latency, AllGather 1x.
            cc_in = dpool.tile([O, 2], F32)
            cc_out = dpool.tile([N_CORES * O, 2], F32, addr_space="Shared")
            nc.sync.dma_start(cc_in[:, :], stats[:, :])
            nc.gpsimd.collective_compute(
                "AllGather", ALU.bypass,
                ins=[cc_in[:, :]],
                outs=[cc_out[:, :]],
                replica_groups=[list(range(N_CORES))],
            )
            gall = work.tile([O, 2 * N_CORES], F32)
            nc.sync.dma_start(
                gall[:, :].rearrange("p (two k) -> p two k", two=2, k=N_CORES),
                cc_out[:, :].rearrange("(k p) two -> p two k", k=N_CORES, p=O))
            gstats = work.tile([O, 2], F32)
            nc.vector.reduce_sum(
                gstats[:, 0:2],
                gall[:, :].rearrange("p (two k) -> p two k", two=2, k=N_CORES),
                axis=mybir.AxisListType.X)

            mean = work.tile([O, 1], F32)
            var = work.tile([O, 1], F32)
            scale = work.tile([O, 1], F32)
            shift = work.tile([O, 1], F32)
            tmp = work.tile([O, 1], F32)
            nc.vector.tensor_scalar(out=mean[:, :], in0=gstats[:, 0:1],
                                    scalar1=1.0 / BN_COUNT, scalar2=None, op0=ALU.mult)
            nc.vector.tensor_scalar(out=var[:, :], in0=gstats[:, 1:2],
                                    scalar1=1.0 / BN_COUNT, scalar2=None, op0=ALU.mult)
            nc.vector.tensor_tensor(out=tmp[:, :], in0=mean[:, :], in1=mean[:, :],
                                    op=ALU.mult)
            nc.vector.tensor_tensor(out=var[:, :], in0=var[:, :], in1=tmp[:, :],
                                    op=ALU.subtract)
            nc.vector.tensor_scalar(out=var[:, :], in0=var[:, :], scalar1=BN_EPS,
                                    scalar2=None, op0=ALU.add)
            nc.scalar.activation(tmp[:, :], var[:, :], AF.Sqrt)
            nc.vector.reciprocal(out=tmp[:, :], in_=tmp[:, :])
            nc.vector.tensor_tensor(out=scale[:, :], in0=vecs_sb[:, 1:2],
                                    in1=tmp[:, :], op=ALU.mult)
            nc.vector.tensor_tensor(out=tmp[:, :], in0=mean[:, :], in1=scale[:, :],
                                    op=ALU.mult)
            nc.vector.tensor_tensor(out=shift[:, :], in0=vecs_sb[:, 2:3],
                                    in1=tmp[:, :], op=ALU.subtract)

            hg = work.tile([O, 512], F32R, tag="hg", bufs=3)
            y_sb = work.tile([O, 512], F32, tag="ysb", bufs=3)
            for c in range(NCH):
                sl = slice(c * 512, (c + 1) * 512)
                hg_t = work.tile([O, 512], F32R, tag="hg", bufs=3, name=f"hg_{c}")
                nc.scalar.activation(hg_t[:, :], h1c[c][:, :], AF.Gelu,
                                     scale=scale[:, :], bias=shift[:, :])
                o_ps = ps_v.tile([O, 512], F32, tag="v_ps", name=f"ops_{c}")
                nc.tensor.matmul(o_ps[:, :], w2r[:, :], hg_t[:, :],
                                 start=True, stop=True)
                y_t = work.tile([O, 512], F32, tag="ysb", bufs=3, name=f"y_{c}")
                nc.vector.tensor_scalar(out=y_t[:, :], in0=o_ps[:, :],
                                        scalar1=vecs_sb[:, 3:4], scalar2=None,
                                        op0=ALU.add)
                nc.sync.dma_start(y_d[:, sl], y_t[:, :])

            for cm in reversed(_cms):
                cm.__exit__(None, None, None)

    if not nc.is_finalized():
        nc.finalize()
    return nc


def _get_runner():
    """Build the Bass module once and cache a jitted 8-core executable."""
    if "runner" in _cache:
        return _cache["runner"]

    import jax
    import concourse.mybir as mb
    from jax.sharding import Mesh, PartitionSpec
    from jax.experimental.shard_map import shard_map
    from concourse import bass2jax

    nc = _build()
    bass2jax.install_neuronx_cc_hook()

    partition_name = nc.partition_id_tensor.name if nc.partition_id_tensor else None
    in_names = []
    out_names = []
    out_avals = []
    for alloc in nc.m.functions[0].allocations:
        if not isinstance(alloc, mb.MemoryLocationSet):
            continue
        name = alloc.memorylocations[0].name
        if alloc.kind == "ExternalInput":
            if name != partition_name:
                in_names.append(name)
        elif alloc.kind == "ExternalOutput":
            out_names.append(name)
            out_avals.append(
                jax.core.ShapedArray(tuple(alloc.tensor_shape), mb.dt.np(alloc.dtype))
            )
    n_params = len(in_names)
    all_in_names = list(in_names)
    if partition_name is not None:
        all_in_names = all_in_names + [partition_name]

    def _body(*args):
        operands = list(args)
        if partition_name is not None:
            operands.append(bass2jax.partition_id_tensor())
        outs = bass2jax._bass_exec_p.bind(
            *operands,
            out_avals=tuple(out_avals),
            in_names=tuple(all_in_names),
            out_names=tuple(out_names),
            lowering_input_output_aliases=(),
            sim_require_finite=True,
            sim_require_nnan=True,
            nc=nc,
        )
        return tuple(outs)

    devices = jax.devices()[:N_CORES]
    assert len(devices) == N_CORES, f"need {N_CORES} devices, have {len(jax.devices())}"
    mesh = Mesh(np.asarray(devices), ("core",))
    n_outs = len(out_names)
    sharded = jax.jit(
        shard_map(
            _body,
            mesh=mesh,
            in_specs=(PartitionSpec("core"),) * n_params,
            out_specs=(PartitionSpec("core"),) * n_outs,
            check_rep=False,
        ),
        keep_unused=True,
    )
    _cache["runner"] = (sharded, in_names, out_names, out_avals)
    return _cache["runner"]


def kernel(**inputs) -> np.ndarray:
    x = np.asarray(inputs["x"], dtype=np.float32)
    assert x.shape == (B, C, N, 1), x.shape
    k = int(np.asarray(inputs.get("k", K_NN)))
    assert k == K_NN, f"kernel compiled for k={K_NN}, got {k}"
    w1 = np.asarray(inputs["w1"], dtype=np.float32)
    b1 = np.asarray(inputs["b1"], dtype=np.float32)
    gamma = np.asarray(inputs["gamma"], dtype=np.float32)
    beta = np.asarray(inputs["beta"], dtype=np.float32)
    w2 = np.asarray(inputs["w2"], dtype=np.float32)
    b2 = np.asarray(inputs["b2"], dtype=np.float32)
    eps_gin = float(np.asarray(inputs["eps_gin"]))

    sharded, in_names, out_names, out_avals = _get_runner()

    xb = np.ascontiguousarray(x[:, :, :, 0])                     # [B, C, N]
    vecs = np.stack(
        [b1, gamma, beta, b2, np.full(O, 1.0 + eps_gin, np.float32)], axis=1
    ).astype(np.float32)                                         # [64, 5]
    ones2 = np.ones((2, N), np.float32)
    ones_col = np.ones((C, 1), np.float32)
    identr = np.eye(C, dtype=np.float32)

    per_core = {
        "xb": xb,
        "w1": np.broadcast_to(w1, (N_CORES,) + w1.shape),
        "w2": np.broadcast_to(w2, (N_CORES,) + w2.shape),
        "vecs": np.broadcast_to(vecs, (N_CORES,) + vecs.shape),
        "ones2": np.broadcast_to(ones2, (N_CORES,) + ones2.shape),
        "ones_col": np.broadcast_to(ones_col, (N_CORES,) + ones_col.shape),
        "identr": np.broadcast_to(identr, (N_CORES,) + identr.shape),
    }
    concat_in = [
        np.ascontiguousarray(per_core[name]).reshape(
            (N_CORES * per_core[name].shape[1],) + per_core[name].shape[2:]
        )
        for name in in_names
    ]
    out_arrs = sharded(*concat_in)
    yi = out_names.index("y")
    y = np.asarray(out_arrs[yi]).reshape(N_CORES, O, N)
    return y[..., None].astype(np.float32)


# revision 32
# speedup vs baseline: 2.0364x; 1.0384x over previous
"""DyGraphGIN2d Trainium kernel: kNN graph (k=16) + GIN aggregation + MLP/BN/GELU.

Sharding: data-parallel over batch B=8 across 8 NeuronCores (one batch
element per core). BatchNorm uses training-mode batch statistics over ALL
B*N rows, so per-core partial sums go through one in-kernel AllReduce.

Algorithm per core (N=4096 points, C=64 dims), single-matmul phases:
  The ranking metric s[n,m] = <x_hi_n, x_hi_m> + q_m with q = -|x|^2/2
  carried EXACTLY as two f32r rank-1 rows (q_hi + q_lo, an exact hi/lo
  split), all folded into ONE K=66 f32r matmul per 128x512 block (extra
  contraction rows are free: matmul cost is output-columns only).
  Phase 1 (tau): per 128-row stripe, 8 such matmuls + DVE top-8 per
  512-chunk + max/match_replace/max give the 16th-largest s per row;
  tau rides back into the XL operand as two more exact f32r rank-1 rows
  (-tau_hi, -tau_lo).
  Phase 2 (mask+aggregate): v' = s^T - tau is recomputed transposed by the
  mirrored K=68 matmul -- the first 66 product terms are bitwise identical
  to phase 1 (commuted multiplies, same PSUM order), so selection is
  bit-consistent; the 1e-5 guard inside tau makes v' > 0 strict for the
  16 selected neighbors. Masks {0,1} are made OFF the DVE: ACT computes
  sigmoid(4e6 * v') (saturates to exactly 1.0/0.0 in f32r) and Pool(GPSIMD)
  computes is_gt(v', 0), alternating per block. aggr[c,n] accumulates in
  PSUM via single-pass f32r matmuls with x_hi^T (PE-transposed).
  The phase-1 stripes (DVE-bound) and phase-2 blocks (PE-bound) are
  software-pipelined chunk-by-chunk so PE/DVE/ACT/Pool all stay busy.
  Tail: h = (1+eps)x + aggr (Pool); h1 = W1^T h + b1 (f32r PE + ACT bias
  with accum_out BN sums); BN stats AllReduce; fused BN+erf-GELU on ACT;
  out = W2^T hg + b2.

The jitted 8-core shard_map executable is cached across kernel() calls.
"""

import numpy as np

import concourse.bacc as bacc
import concourse.mybir as mybir
from concourse.tile import TileContext

F32 = mybir.dt.float32
F32R = mybir.dt.float32r
AF = mybir.ActivationFunctionType
ALU = mybir.AluOpType

B, C, N, O = 8, 64, 4096, 64
K_NN = 16
N_CORES = 8
NT = N // 128          # 32 row stripes
NCH = 8                # 512-wide column chunks
BN_EPS = 1e-5
BN_COUNT = float(B * N)
TAU_GUARD = 1e-5
SIG_SCALE = 4e6

_cache = {}


def _build():
    nc = bacc.Bacc("TRN2", target_bir_lowering=False)

    xb_d = nc.dram_tensor("xb", [C, N], F32, kind="ExternalInput")
    w1_d = nc.dram_tensor("w1", [C, O], F32, kind="ExternalInput")
    w2_d = nc.dram_tensor("w2", [O, O], F32, kind="ExternalInput")
    vecs_d = nc.dram_tensor("vecs", [O, 5], F32, kind="ExternalInput")  # b1,gamma,beta,b2,eps1
    ones2_d = nc.dram_tensor("ones2", [2, N], F32R, kind="ExternalInput")
    identr_d = nc.dram_tensor("identr", [C, C], F32R, kind="ExternalInput")
    y_d = nc.dram_tensor("y", [O, N], F32, kind="ExternalOutput")
    tau_scr = nc.dram_tensor("tau_scr", [N, 2], F32R)  # internal scratch

    with TileContext(nc) as tc:
        with tc.tile_pool(name="big", bufs=1) as big, \
             tc.tile_pool(name="work", bufs=1) as work, \
             tc.tile_pool(name="dram", bufs=1, space="DRAM") as dpool:

            # ---------------- prologue: operands ----------------
            vecs_sb = work.tile([O, 5], F32)
            w1_sb = work.tile([C, O], F32)
            w2_sb = work.tile([O, O], F32)
            identr = work.tile([C, C], F32R)
            w1r = work.tile([C, O], F32R)
            w2r = work.tile([O, O], F32R)
            eid = work.tile([C, C], F32R)

            xbc = [big.tile([C, 512], F32, name=f"xbc{i}") for i in range(NCH)]
            # XLc: p1 lhsT rows [x_hi; 1; 1], p2 rhs rows [x_hi; 1; 1;
            # -tau_hi; -tau_lo].  The tau rows are DMA-written only after all
            # p1 stripes of the chunk have been emitted, so no false
            # whole-tile WAR stalls arise.
            XLc = [big.tile([128, 512], F32R, name=f"XLc{i}") for i in range(NCH)]
            XRc = [big.tile([128, 512], F32R, name=f"XRc{i}") for i in range(NCH)]
            xt_sb = big.tile([128, NT * C], F32R)

            import concourse.bass_isa as bass_isa
            xsq = work.tile([C, 512], F32, tag="xsq", bufs=2)
            sqall = work.tile([C, 512], F32, tag="sqall", bufs=2)
            qt = work.tile([1, 512], F32, tag="qt", bufs=2)
            # xb loads first: they head the per-chunk critical chains and the
            # single HWDGE queue drains in emission order.
            for c in range(NCH):
                nc.sync.dma_start(xbc[c][:, :], xb_d[:, c * 512 : (c + 1) * 512])
            nc.sync.dma_start(vecs_sb[:, :], vecs_d[:, :])
            nc.sync.dma_start(w1_sb[:, :], w1_d[:, :])
            nc.sync.dma_start(w2_sb[:, :], w2_d[:, :])
            nc.sync.dma_start(identr[:, :], identr_d[:, :])
            nc.scalar.activation(w1r[:, :], w1_sb[:, :], AF.Copy)
            nc.scalar.activation(w2r[:, :], w2_sb[:, :], AF.Copy)
            # diag(1+eps) in f32r: folds the (1+eps)*x_hi term into the
            # aggregation PSUM via one extra matmul per chunk.
            nc.scalar.activation(eid[:, :], identr[:, :], AF.Copy,
                                 scale=vecs_sb[:, 4:5])
            for c in range(NCH):
                sl = slice(c * 512, (c + 1) * 512)
                nc.scalar.activation(XLc[c][:C, :], xbc[c][:, :], AF.Copy)
                nc.scalar.dma_start(XLc[c][C : C + 2, :], ones2_d[:, sl])
                nc.gpsimd.tensor_copy(XRc[c][:C, :], XLc[c][:C, :])
                nc.scalar.dma_start(XRc[c][C + 2 : C + 4, :], ones2_d[:, sl])
                # q = -|x_m|^2/2 exactly as f32r hi+lo rank-1 rows; the
                # partition reduce runs on the idle GPSIMD engine (cold-PE
                # fp32 matmuls here cost ~2.4us each at low p-state).
                xsq_t = work.tile([C, 512], F32, tag="xsq", bufs=2, name=f"xsq_{c}")
                sq_t = work.tile([C, 512], F32, tag="sqall", bufs=2, name=f"sqa_{c}")
                nc.gpsimd.tensor_tensor(out=xsq_t[:, :], in0=xbc[c][:, :],
                                        in1=xbc[c][:, :], op=ALU.mult)
                nc.gpsimd.partition_all_reduce(sq_t[:, :], xsq_t[:, :],
                                               channels=C,
                                               reduce_op=bass_isa.ReduceOp.add)
                nc.scalar.activation(XRc[c][C : C + 1, :], sq_t[0:1, :], AF.Copy,
                                     scale=-0.5)
                qh_t = work.tile([1, 512], F32R, tag="qh", bufs=2, name=f"qh_{c}")
                nc.scalar.activation(qh_t[:, :], sq_t[0:1, :], AF.Copy, scale=-0.5)
                qt_t = work.tile([1, 512], F32, tag="qt", bufs=2, name=f"qt_{c}")
                nc.vector.tensor_scalar(out=qt_t[:, :], in0=sq_t[0:1, :],
                                        scalar1=-0.5, scalar2=None, op0=ALU.mult)
                nc.vector.tensor_tensor(out=qt_t[:, :], in0=qt_t[:, :],
                                        in1=qh_t.bitcast(F32)[:, :],
                                        op=ALU.subtract)
                # ACT can only write at partition base 0/64; q_lo (row 65)
                # goes through a partition-0 staging tile + DMA.
                ql_t = work.tile([1, 512], F32R, tag="ql", bufs=2,
                                 name=f"ql_{c}")
                nc.scalar.activation(ql_t[:, :], qt_t[:, :], AF.Copy)
                nc.sync.dma_start(XRc[c][C + 1 : C + 2, :], ql_t[:, :])

            # ---------------- main pipelined loop ----------------
            # ps_v/ps_a are entered only after the transpose block below so
            # its PSUM fits; ps_s is needed from iteration 0.
            ps_s_cm = tc.tile_pool(name="ps_s", bufs=3, space="PSUM")
            ps_s = ps_s_cm.__enter__()
            ps_v = ps_a = None
            _cms = [ps_s_cm]

            cand = work.tile([128, 64], F32, tag="cand", bufs=5)
            t8a = work.tile([128, 8], F32, tag="t8a", bufs=4)
            t8b = work.tile([128, 8], F32, tag="t8b", bufs=4)
            ntf = work.tile([128, 1], F32, tag="ntf", bufs=4)
            ntau2 = work.tile([128, 2], F32R, tag="ntau2", bufs=4)
            mask = work.tile([128, 512], F32R, tag="mask", bufs=6)
            hc = work.tile([C, 512], F32R, tag="hc", bufs=3)
            h1c = [big.tile([O, 512], F32, name=f"h1c{i}") for i in range(NCH)]
            bnsum = work.tile([O, NCH], F32)
            bnsq = work.tile([O, NCH], F32)
            sqscr = work.tile([O, 512], F32, tag="sqscr", bufs=2)
            eps1 = vecs_sb[:, 4:5]

            aggr_tiles = {}

            def p1_mm(s, c8, cand_t):
                jt, jo = s // 4, (s % 4) * 128
                s_ps = ps_s.tile([128, 512], F32, tag="s_ps", name=f"s_{s}_{c8}")
                nc.tensor.matmul(s_ps[:, :], XLc[jt][: C + 2, jo : jo + 128],
                                 XRc[c8][: C + 2, :], start=True, stop=True)
                nc.vector.max(out=cand_t[:, c8 * 8 : (c8 + 1) * 8], in_=s_ps[:, :])

            def p1_tail(s, cand_t):
                jt, jo = s // 4, (s % 4) * 128
                t8a_t = work.tile([128, 8], F32, tag="t8a", bufs=4, name=f"t8a_{s}")
                t8b_t = work.tile([128, 8], F32, tag="t8b", bufs=4, name=f"t8b_{s}")
                ntf_t = work.tile([128, 1], F32, tag="ntf", bufs=4, name=f"ntf_{s}")
                nt2_t = work.tile([128, 2], F32R, tag="ntau2", bufs=4, name=f"nt2_{s}")
                nc.vector.max(out=t8a_t[:, :], in_=cand_t[:, :])
                nc.vector.match_replace(out=cand_t[:, :], in_to_replace=t8a_t[:, :],
                                        in_values=cand_t[:, :], imm_value=-1e30)
                nc.vector.max(out=t8b_t[:, :], in_=cand_t[:, :])
                # -tau = -(t16 - guard) = guard - t16, split exactly hi+lo
                nc.gpsimd.tensor_scalar(out=ntf_t[:, :], in0=t8b_t[:, 7:8],
                                        scalar1=-1.0, scalar2=TAU_GUARD,
                                        op0=ALU.mult, op1=ALU.add)
                nc.vector.tensor_copy(nt2_t[:, 0:1], ntf_t[:, :])
                nc.gpsimd.tensor_tensor(out=nt2_t.bitcast(F32)[:, 1:2],
                                        in0=ntf_t[:, :],
                                        in1=nt2_t.bitcast(F32)[:, 0:1],
                                        op=ALU.subtract)
                # SBUF->SBUF DMA cannot transpose partition->free; bounce
                # the per-stripe [128,2] tau pair through flat DRAM.  The
                # chunk-wide readback into XLc happens in tau_readback().
                nc.sync.dma_start(tau_scr[s * 128 : (s + 1) * 128, :], nt2_t[:, 0:2])

            def p2_v(c, j):
                jt, jo = j // 4, (j % 4) * 128
                v_ps = ps_v.tile([128, 512], F32, tag="v_ps", name=f"v_{c}_{j}")
                nc.tensor.matmul(v_ps[:, :], XRc[jt][: C + 4, jo : jo + 128],
                                 XLc[c][: C + 4, :], start=True, stop=True)
                m = work.tile([128, 512], F32R, tag="mask", bufs=6, name=f"m_{c}_{j}")
                if c == NCH - 1 and j % 2 == 0:
                    nc.vector.tensor_scalar(out=m[:, :], in0=v_ps[:, :],
                                            scalar1=0.0, scalar2=None,
                                            op0=ALU.is_gt)
                else:
                    nc.scalar.activation(m[:, :], v_ps[:, :], AF.Sigmoid,
                                         scale=SIG_SCALE)
                return m

            def p2_aggr(c, j, m):
                nc.tensor.matmul(aggr_tiles[c][:, :],
                                 xt_sb[:, j * C : (j + 1) * C],
                                 m[:, :],
                                 start=(j == 0), stop=False)

            def finish_mlp(c):
                sl = slice(c * 512, (c + 1) * 512)
                nc.tensor.matmul(aggr_tiles[c][:, :], eid[:, :], XLc[c][:C, :],
                                 start=False, stop=True)
                h_t = work.tile([C, 512], F32R, tag="hc", bufs=3, name=f"h_{c}")
                nc.scalar.activation(h_t[:, :], aggr_tiles[c][:, :], AF.Copy)
                h1_ps = ps_v.tile([O, 512], F32, tag="v_ps", name=f"h1ps_{c}")
                nc.tensor.matmul(h1_ps[:, :], w1r[:, :], h_t[:, :],
                                 start=True, stop=True)
                nc.scalar.activation(h1c[c][:, :], h1_ps[:, :], AF.Identity,
                                     bias=vecs_sb[:, 0:1],
                                     accum_out=bnsum[:, c : c + 1])
                sq_t = work.tile([O, 512], F32, tag="sqscr", bufs=2, name=f"sq_{c}")
                nc.scalar.activation(sq_t[:, :], h1c[c][:, :], AF.Square,
                                     accum_out=bnsq[:, c : c + 1])

            # software pipeline: iteration it runs phase-1 stripes of chunk
            # it and phase-2 of chunk it-1, interleaved 1:1 on the PE stream.
            for it in range(NCH + 1):
                if it == 1:
                    # x_hi^T chunks for the aggregation matmuls: emitted here
                    # so the PE transposes + ACT copies overlap iteration 0's
                    # DVE-only top-8 work.
                    with tc.tile_pool(name="ps_tp", bufs=2, space="PSUM") as ps_tp:
                        for j in range(NT):
                            tp = ps_tp.tile([128, C], F32R, tag="tp_ps")
                            nc.tensor.transpose(
                                tp[:, :],
                                XLc[j // 4][:C, (j % 4) * 128 : (j % 4 + 1) * 128],
                                identr[:, :])
                            nc.scalar.activation(xt_sb[:, j * C : (j + 1) * C],
                                                 tp[:, :], AF.Copy)
                    ps_v_cm = tc.tile_pool(name="ps_v", bufs=3, space="PSUM")
                    ps_a_cm = tc.tile_pool(name="ps_a", bufs=2, space="PSUM")
                    ps_v = ps_v_cm.__enter__()
                    ps_a = ps_a_cm.__enter__()
                    _cms.extend([ps_v_cm, ps_a_cm])
                c1 = it if it < NCH else None
                c2 = it - 1 if it >= 1 else None
                if c2 is not None:
                    aggr_tiles[c2] = ps_a.tile([O, 512], F32, tag="aggr",
                                               name=f"aggr_{c2}")
                cands = {}
                masks = {}
                if c1 is not None:
                    for s in range(4 * c1, 4 * c1 + 4):
                        cands[s] = work.tile([128, 64], F32, tag="cand",
                                             bufs=5, name=f"cand_{s}")
                for k in range(NT):
                    if c1 is not None:
                        if c2 is None:
                            # iteration 0: chunk-major order so the early
                            # chunks' top-8 passes run while the later
                            # prologue chunks are still being prepared.
                            s, c8 = 4 * c1 + k % 4, k // 4
                        else:
                            s, c8 = 4 * c1 + k // 8, k % 8
                        p1_mm(s, c8, cands[s])
                    if c2 is not None:
                        masks[k] = p2_v(c2, k)
                        if k >= 2:
                            p2_aggr(c2, k - 2, masks.pop(k - 2))
                if c1 is not None:
                    # tails only after every stripe's XLc reads are emitted:
                    # the tau DMAs then order cleanly behind them.
                    for kk in range(4):
                        s = 4 * c1 + kk
                        p1_tail(s, cands[s])
                    nc.sync.dma_start(
                        XLc[c1][C + 2 : C + 4, :],
                        tau_scr[c1 * 512 : (c1 + 1) * 512, 0:2].rearrange(
                            "p two -> two p"))
                if c2 is not None:
                    p2_aggr(c2, NT - 2, masks.pop(NT - 2))
                    p2_aggr(c2, NT - 1, masks.pop(NT - 1))
                    finish_mlp(c2)

            # ---------------- BN combine + AllReduce + GELU + W2 ---------
            stats = work.tile([O, 2], F32)
            nc.vector.reduce_sum(stats[:, 0:1], bnsum[:, :], axis=mybir.AxisListType.X)
            nc.vector.reduce_sum(stats[:, 1:2], bnsq[:, :], axis=mybir.AxisListType.X)

            # AllGather + local 8-way add: the collective cost model charges
            # AllReduce 1.875x the fixed ~15us latency, AllGather 1x.
            cc_in = dpool.tile([O, 2], F32)
            cc_out = dpool.tile([N_CORES * O, 2], F32, addr_space="Shared")
            nc.sync.dma_start(cc_in[:, :], stats[:, :])
            nc.gpsimd.collective_compute(
                "AllGather", ALU.bypass,
                ins=[cc_in[:, :]],
                outs=[cc_out[:, :]],
                replica_groups=[list(range(N_CORES))],
            )
            gall = work.tile([O, 2 * N_CORES], F32)
            nc.sync.dma_start(
                gall[:, :].rearrange("p (two k) -> p two k", two=2, k=N_CORES),
                cc_out[:, :].rearrange("(k p) two -> p two k", k=N_CORES, p=O))
            gstats = work.tile([O, 2], F32)
            nc.vector.reduce_sum(
                gstats[:, 0:2],
                gall[:, :].rearrange("p (two k) -> p two k", two=2, k=N_CORES),
                axis=mybir.AxisListType.X)

            mean = work.tile([O, 1], F32)
            var = work.tile([O, 1], F32)
            scale = work.tile([O, 1], F32)
            shift = work.tile([O, 1], F32)
            tmp = work.tile([O, 1], F32)
            nc.vector.tensor_scalar(out=mean[:, :], in0=gstats[:, 0:1],
                                    scalar1=1.0 / BN_COUNT, scalar2=None, op0=ALU.mult)
            nc.vector.tensor_scalar(out=var[:, :], in0=gstats[:, 1:2],
                                    scalar1=1.0 / BN_COUNT, scalar2=None, op0=ALU.mult)
            nc.vector.tensor_tensor(out=tmp[:, :], in0=mean[:, :], in1=mean[:, :],
                                    op=ALU.mult)
            nc.vector.tensor_tensor(out=var[:, :], in0=var[:, :], in1=tmp[:, :],
                                    op=ALU.subtract)
            nc.vector.tensor_scalar(out=var[:, :], in0=var[:, :], scalar1=BN_EPS,
                                    scalar2=None, op0=ALU.add)
            nc.scalar.activation(tmp[:, :], var[:, :], AF.Sqrt)
            nc.vector.reciprocal(out=tmp[:, :], in_=tmp[:, :])
            nc.vector.tensor_tensor(out=scale[:, :], in0=vecs_sb[:, 1:2],
                                    in1=tmp[:, :], op=ALU.mult)
            nc.vector.tensor_tensor(out=tmp[:, :], in0=mean[:, :], in1=scale[:, :],
                                    op=ALU.mult)
            nc.vector.tensor_tensor(out=shift[:, :], in0=vecs_sb[:, 2:3],
                                    in1=tmp[:, :], op=ALU.subtract)

            hg = work.tile([O, 512], F32R, tag="hg", bufs=3)
            y_sb = work.tile([O, 512], F32, tag="ysb", bufs=3)
            for c in range(NCH):
                sl = slice(c * 512, (c + 1) * 512)
                hg_t = work.tile([O, 512], F32R, tag="hg", bufs=3, name=f"hg_{c}")
                nc.scalar.activation(hg_t[:, :], h1c[c][:, :], AF.Gelu,
                                     scale=scale[:, :], bias=shift[:, :])
                o_ps = ps_v.tile([O, 512], F32, tag="v_ps", name=f"ops_{c}")
                nc.tensor.matmul(o_ps[:, :], w2r[:, :], hg_t[:, :],
                                 start=True, stop=True)
                y_t = work.tile([O, 512], F32, tag="ysb", bufs=3, name=f"y_{c}")
                nc.vector.tensor_scalar(out=y_t[:, :], in0=o_ps[:, :],
                                        scalar1=vecs_sb[:, 3:4], scalar2=None,
                                        op0=ALU.add)
                nc.sync.dma_start(y_d[:, sl], y_t[:, :])

            for cm in reversed(_cms):
                cm.__exit__(None, None, None)

    if not nc.is_finalized():
        nc.finalize()
    return nc


def _get_runner():
    """Build the Bass module once and cache a jitted 8-core executable."""
    if "runner" in _cache:
        return _cache["runner"]

    import jax
    import concourse.mybir as mb
    from jax.sharding import Mesh, PartitionSpec
    from jax.experimental.shard_map import shard_map
    from concourse import bass2jax

    nc = _build()
    bass2jax.install_neuronx_cc_hook()

    partition_name = nc.partition_id_tensor.name if nc.partition_id_tensor else None
    in_names = []
    out_names = []
    out_avals = []
    for alloc in nc.m.functions[0].allocations:
        if not isinstance(alloc, mb.MemoryLocationSet):
            continue
        name = alloc.memorylocations[0].name
        if alloc.kind == "ExternalInput":
            if name != partition_name:
                in_names.append(name)
        elif alloc.kind == "ExternalOutput":
            out_names.append(name)
            out_avals.append(
                jax.core.ShapedArray(tuple(alloc.tensor_shape), mb.dt.np(alloc.dtype))
            )
    n_params = len(in_names)
    all_in_names = list(in_names)
    if partition_name is not None:
        all_in_names = all_in_names + [partition_name]

    def _body(*args):
        operands = list(args)
        if partition_name is not None:
            operands.append(bass2jax.partition_id_tensor())
        outs = bass2jax._bass_exec_p.bind(
            *operands,
            out_avals=tuple(out_avals),
            in_names=tuple(all_in_names),
            out_names=tuple(out_names),
            lowering_input_output_aliases=(),
            sim_require_finite=True,
            sim_require_nnan=True,
            nc=nc,
        )
        return tuple(outs)

    devices = jax.devices()[:N_CORES]
    assert len(devices) == N_CORES, f"need {N_CORES} devices, have {len(jax.devices())}"
    mesh = Mesh(np.asarray(devices), ("core",))
    n_outs = len(out_names)
    sharded = jax.jit(
        shard_map(
            _body,
            mesh=mesh,
            in_specs=(PartitionSpec("core"),) * n_params,
            out_specs=(PartitionSpec("core"),) * n_outs,
            check_rep=False,
        ),
        keep_unused=True,
    )
    _cache["runner"] = (sharded, in_names, out_names, out_avals)
    return _cache["runner"]


def kernel(**inputs) -> np.ndarray:
    x = np.asarray(inputs["x"], dtype=np.float32)
    assert x.shape == (B, C, N, 1), x.shape
    k = int(np.asarray(inputs.get("k", K_NN)))
    assert k == K_NN, f"kernel compiled for k={K_NN}, got {k}"
    w1 = np.asarray(inputs["w1"], dtype=np.float32)
    b1 = np.asarray(inputs["b1"], dtype=np.float32)
    gamma = np.asarray(inputs["gamma"], dtype=np.float32)
    beta = np.asarray(inputs["beta"], dtype=np.float32)
    w2 = np.asarray(inputs["w2"], dtype=np.float32)
    b2 = np.asarray(inputs["b2"], dtype=np.float32)
    eps_gin = float(np.asarray(inputs["eps_gin"]))

    sharded, in_names, out_names, out_avals = _get_runner()

    xb = np.ascontiguousarray(x[:, :, :, 0])                     # [B, C, N]
    vecs = np.stack(
        [b1, gamma, beta, b2, np.full(O, 1.0 + eps_gin, np.float32)], axis=1
    ).astype(np.float32)                                         # [64, 5]
    ones2 = np.ones((2, N), np.float32)
    ones_col = np.ones((C, 1), np.float32)
    identr = np.eye(C, dtype=np.float32)

    per_core = {
        "xb": xb,
        "w1": np.broadcast_to(w1, (N_CORES,) + w1.shape),
        "w2": np.broadcast_to(w2, (N_CORES,) + w2.shape),
        "vecs": np.broadcast_to(vecs, (N_CORES,) + vecs.shape),
        "ones2": np.broadcast_to(ones2, (N_CORES,) + ones2.shape),
        "ones_col": np.broadcast_to(ones_col, (N_CORES,) + ones_col.shape),
        "identr": np.broadcast_to(identr, (N_CORES,) + identr.shape),
    }
    concat_in = [
        np.ascontiguousarray(per_core[name]).reshape(
            (N_CORES * per_core[name].shape[1],) + per_core[name].shape[2:]
        )
        for name in in_names
    ]
    out_arrs = sharded(*concat_in)
    yi = out_names.index("y")
    y = np.asarray(out_arrs[yi]).reshape(N_CORES, O, N)
    return y[..., None].astype(np.float32)
